# revision 6
# baseline (speedup 1.0000x reference)
"""Trainium2 Bass kernel for nn_DistiledMultiheadAttention_76476187673064.

Sliding-window (W=32) single-query attention over ragged sequences with a
learned pre-context buffer, plus input/output projections.

Strategy (8 NeuronCores, data-parallel over flat tokens):
  - Each core owns 512 tokens; kv for a 31-token halo is recomputed locally
    (plus one masked pad column), so no collectives are needed.
  - All matmul operands are bf16 (host-cast, fp32 PSUM accumulation).
  - HAM-warmth-driven schedule: the attention QK/PV matmuls have low PE
    array activity (32/128 contraction rows resp. 65/128 output columns),
    which keeps the PE clock-gated at 1.2GHz if they run as a contiguous
    phase.  This version interleaves the dense full-array work (V
    projection, output projection, normalization) INTO the attention
    iterations so every HAM activity window sees high utilization and the
    PE stays at 2.4GHz.
  - Startup: x and Wk are split into per-e-chunk DMAs issued concurrently
    from the Sync AND Activation HWDGE queues; K projection runs e-outer
    (4 PSUM accumulators live) so matmuls start as soon as the first
    chunks land instead of after the full tensors.
  - Additive band/segment/buffer masks are host-precomputed in bf16
    (half the HBM traffic); exp without max-subtraction (logits bounded).
  - ScalarE runs ONLY Exp + Copy (single ACT table load); softmax sums
    are gathered per block and reciprocated by one DVE InstReciprocal;
    normalization applied as rank-2 broadcast matmuls into the attention
    output ahead of the output projection.
"""
import math
import sys

sys.path.insert(0, "/opt/trn_rl_repo")

import numpy as np

# ---------------------------------------------------------------- constants
T = 4096
E = 1024
KD = 512          # key dim
H = 16            # heads
W = 32            # window
DK = KD // H      # 32
DV = E // H       # 64
B = 8
MAXL = 768
N_CORES = 8
SHARD = T // N_CORES          # 512 tokens per core
HALO = W - 1                  # 31
NTOK = SHARD + HALO + 1       # 544 token columns incl. halo + 1 pad
TB = 256                      # attention token block
NB = SHARD // TB              # 2 blocks per core
NEG = -30000.0

_CACHE = {}


# ------------------------------------------------------------- tile patches
def _apply_tile_patches():
    """This container's walrus only supports ONE sync-wait per instruction;
    redistribute extra Tile-assigned waits onto single-wait InstNoOp carriers."""
    import concourse.mybir as mybir
    import concourse.tile as tile
    from concourse.vector_clock import ScopedClock

    if getattr(tile.TileContext, "_wait_split_patched", False):
        return
    orig_commit = tile.TileContext._commit_and_lower

    def commit_split(self, inst, original_block, old_bb_map, bb_to_exit_bb):
        si = getattr(inst, "sync_info", None)
        if si is not None and si.on_wait and len(si.on_wait) > 1:
            engine = inst.engine
            if engine is not None and engine != mybir.EngineType.Unassigned:
                waits = list(si.on_wait)
                si.on_wait = waits[-1:]
                for w in waits[:-1]:
                    noop = mybir.InstNoOp(
                        name=self.nc.get_next_instruction_name(),
                        sync_info=mybir.SyncInfo(on_wait=[w], on_update=[]),
                        bass_nofuse=True,
                        engine=engine,
                        text_hint="wait_split",
                    )
                    orig_commit(self, noop, original_block, old_bb_map, bb_to_exit_bb)
        return orig_commit(self, inst, original_block, old_bb_map, bb_to_exit_bb)

    def drain_and_barrier(self, tick_clock, wait_clock):
        drain_inst = self.nc.sync.drain()
        wait_clock.add_sem_waits(
            drain_inst.ins, ScopedClock({None: tick_clock.global_clock})
        )
        si = drain_inst.ins.sync_info
        if si is not None and si.on_wait and len(si.on_wait) > 1:
            waits = list(si.on_wait)
            si.on_wait = waits[:1]
            for w in waits[1:]:
                nop = self.nc.sync.nop(nofuse=True)
                nsi = nop.ins.sync_info
                if nsi is None:
                    nop.ins.sync_info = mybir.SyncInfo(on_wait=[w], on_update=[])
                else:
                    nsi.on_wait = list(nsi.on_wait or []) + [w]
        self.nc.all_engine_barrier()
        assert self.sems is not None
        popped = self.nc._tile_sem_poison_stack.pop()
        assert popped is self._sem_poison
        self.nc.clear_and_free_semaphores(list(self.sems.allocated().values()))
        self.nc.all_engine_barrier()

    tile.TileContext._commit_and_lower = commit_split
    tile.TileContext._drain_and_barrier = drain_and_barrier
    tile.TileContext._wait_split_patched = True


# ------------------------------------------------------------- device build
def _build_nc(with_bias=True):
    import concourse.bass as bass
    import concourse.mybir as mybir
    import concourse.tile as tile

    _apply_tile_patches()
    f32 = mybir.dt.float32
    f32r = mybir.dt.float32r
    bf16 = mybir.dt.bfloat16
    ADD = mybir.AluOpType.add
    MUL = mybir.AluOpType.mult
    EXP = mybir.ActivationFunctionType.Exp

    nc = bass.Bass()
    d_xT = nc.dram_tensor("xT", [128, 8, NTOK], bf16, kind="ExternalInput")
    d_wk = nc.dram_tensor("wk", [128, 8, 4, 128], bf16, kind="ExternalInput")
    d_wq = nc.dram_tensor("wq", [128, 8, 4, 128], bf16, kind="ExternalInput")
    d_wv = nc.dram_tensor("wv", [128, 4, 8, 260], bf16, kind="ExternalInput")
    d_wp = nc.dram_tensor("wp", [128, 2, 8, 512], bf16, kind="ExternalInput")
    d_kbufT = nc.dram_tensor("kbufT", [128, 4, 32], bf16, kind="ExternalInput")
    d_vbuf = nc.dram_tensor("vbuf", [32, 1040], bf16, kind="ExternalInput")
    d_ones = nc.dram_tensor("ones", [1, 128], f32r, kind="ExternalInput")
    d_mask = nc.dram_tensor("mask", [128, NB, 1024], bf16, kind="ExternalInput")
    d_bk = nc.dram_tensor("bk", [128, 4], f32, kind="ExternalInput")
    d_bq = nc.dram_tensor("bq", [128, 4], f32, kind="ExternalInput")
    d_bv = nc.dram_tensor("bv", [1, H * 65], f32r, kind="ExternalInput")
    d_bp = nc.dram_tensor("bp", [1, E], f32r, kind="ExternalInput")
    d_sel2 = nc.dram_tensor("sel2", [2, 128], f32r, kind="ExternalInput")
    d_y = nc.dram_tensor("yout", [SHARD, E], f32, kind="ExternalOutput")
    d_rsc = [nc.dram_tensor(f"rscratch{i}", [16, TB], f32, kind="Internal")
             for i in range(NB)]
    d_ssc = [nc.dram_tensor(f"sscratch{i}", [1, 16 * TB], f32, kind="Internal")
             for i in range(NB)]

    with tile.TileContext(nc) as tc, nc.allow_low_precision(
        reason="bf16 matmul operands; fp32 PSUM accumulation throughout"
    ):
        with (
            tc.tile_pool(name="x", bufs=1) as x_pool,
            tc.tile_pool(name="wgt", bufs=1) as w_pool,
            tc.tile_pool(name="const", bufs=1) as const_pool,
            tc.tile_pool(name="kqv", bufs=1) as kqv_pool,
            tc.tile_pool(name="exp", bufs=12) as exp_pool,
            tc.tile_pool(name="srow", bufs=4) as s_pool,
            tc.tile_pool(name="out", bufs=3) as out_pool,
        ):
            # ---- persistent SBUF tiles
            xT = x_pool.tile([128, 8, NTOK], bf16)
            wk_sb = w_pool.tile([128, 8, 4, 128], bf16)
            wq_sb = w_pool.tile([128, 8, 4, 128], bf16)
            wv_sb = w_pool.tile([128, 4, 8, 260], bf16)
            wp_sb = w_pool.tile([128, 2, 8, 512], bf16)
            kbuf_sb = const_pool.tile([128, 4, 32], bf16)
            vbuf_sb = const_pool.tile([128, 1040], bf16)
            ones = const_pool.tile([1, 128], f32r)
            sel2 = const_pool.tile([2, 128], f32r)
            mask_sb = const_pool.tile([128, NB, 1024], bf16)
            if with_bias:
                bk_sb = const_pool.tile([128, 4], f32)
                bq_sb = const_pool.tile([128, 4], f32)
                bv_sb = const_pool.tile([1, H * 65], f32r)
                bp_sb = const_pool.tile([1, E], f32r)

            # ---- input DMAs, split + spread across both HWDGE engines so
            # K projection can start as soon as the first chunks land.
            # sync: x chunks + everything attention needs early
            # scalar: wk chunks, wv, then late consumers (mask b1, wp)
            for e in range(8):
                nc.sync.dma_start(xT[:, e, :], d_xT[:, e, :])
                nc.scalar.dma_start(wk_sb[:, e, :, :], d_wk[:, e, :, :])
            nc.sync.dma_start(wq_sb[:], d_wq[:])
            nc.scalar.dma_start(wv_sb[:], d_wv[:])
            nc.sync.dma_start(kbuf_sb[:], d_kbufT[:])
            for r in range(4):
                nc.sync.dma_start(vbuf_sb[r * 32:(r + 1) * 32, :], d_vbuf[:])
            nc.sync.dma_start(ones[:], d_ones[:])
            nc.sync.dma_start(sel2[:], d_sel2[:])
            nc.sync.dma_start(mask_sb[:, 0, :], d_mask[:, 0, :])
            if with_bias:
                nc.sync.dma_start(bk_sb[:], d_bk[:])
                nc.sync.dma_start(bq_sb[:], d_bq[:])
                nc.sync.dma_start(bv_sb[:], d_bv[:])
                nc.sync.dma_start(bp_sb[:], d_bp[:])
            nc.scalar.dma_start(mask_sb[:, 1, :], d_mask[:, 1, :])
            nc.scalar.dma_start(wp_sb[:], d_wp[:])

            # ---- persistent activations
            kT = kqv_pool.tile([128, 4, NTOK], bf16)    # K feature-major
            qT = kqv_pool.tile([128, 4, SHARD], bf16)   # Q feature-major (scaled)
            vA = kqv_pool.tile([128, 5, H * 65], bf16)  # V token-major + ones col
            vTail = kqv_pool.tile([128, NB, H * 65], bf16)  # tail V, 4x replicated
            oT = kqv_pool.tile([128, 8, SHARD], bf16)   # attention out feature-major
            s_half = [kqv_pool.tile([16, TB], f32, name=f"s_half{i}")
                      for i in range(NB)]  # sums
            r_half = [kqv_pool.tile([16, TB], f32, name=f"r_half{i}")
                      for i in range(NB)]  # recips
            # recips rearranged: row 0 = even heads, row 1 = odd heads
            r2 = kqv_pool.tile([2, 16 * TB], f32)

            # ================= K/Q projection (e-outer so compute starts
            # as soon as chunk 0 of x and Wk arrive)
            with (
                tc.tile_pool(name="ppa", bufs=4, space="PSUM") as ppa,
                tc.tile_pool(name="ppb", bufs=1, space="PSUM") as ppb,
            ):
                pa = [ppa.tile([128, 512], f32, tag="pa", name=f"pa{m}")
                      for m in range(4)]
                pbt = ppb.tile([128, 4, 32], f32, tag="pb")
                for e in range(8):
                    for m in range(4):
                        nc.tensor.matmul(
                            pa[m][:], wk_sb[:, e, m, :], xT[:, e, 0:512],
                            start=(e == 0), stop=(e == 7),
                        )
                        nc.tensor.matmul(
                            pbt[:, m, :], wk_sb[:, e, m, :], xT[:, e, 512:NTOK],
                            start=(e == 0), stop=(e == 7),
                        )
                for m in range(4):
                    if with_bias:
                        nc.scalar.add(kT[:, m, 0:512], pa[m][:], bk_sb[:, m:m + 1])
                        nc.scalar.add(kT[:, m, 512:NTOK], pbt[:, m, :],
                                      bk_sb[:, m:m + 1])
                    elif m % 2 == 0:
                        nc.scalar.copy(kT[:, m, 0:512], pa[m][:])
                        nc.scalar.copy(kT[:, m, 512:NTOK], pbt[:, m, :])
                    else:
                        nc.vector.tensor_copy(kT[:, m, 0:512], pa[m][:])
                        nc.vector.tensor_copy(kT[:, m, 512:NTOK], pbt[:, m, :])

                # Q projection (tokens only, no halo)
                qa = [ppa.tile([128, 512], f32, tag="pa", name=f"qa{m}")
                      for m in range(4)]
                for e in range(8):
                    for m in range(4):
                        nc.tensor.matmul(
                            qa[m][:], wq_sb[:, e, m, :], xT[:, e, HALO:HALO + SHARD],
                            start=(e == 0), stop=(e == 7),
                        )
                for m in range(4):
                    if with_bias:
                        nc.scalar.add(qT[:, m, :], qa[m][:], bq_sb[:, m:m + 1])
                    elif m % 2 == 0:
                        nc.scalar.copy(qT[:, m, :], qa[m][:])
                    else:
                        nc.vector.tensor_copy(qT[:, m, :], qa[m][:])

            # ================= attention + V projection + output projection,
            # interleaved so the PE array activity stays high (HAM warm)
            with (
                tc.tile_pool(name="pv", bufs=1, space="PSUM") as pv_pool,
                tc.tile_pool(name="plgM", bufs=4, space="PSUM") as plgM,
                tc.tile_pool(name="plgT", bufs=1, space="PSUM") as plgT,
                tc.tile_pool(name="pov", bufs=2, space="PSUM") as pov,
            ):
                tok_sizes = [128, 128, 128, 128, 32]

                def v_unit(f, i):
                    # V projection token-major (wv pre-augmented with zero
                    # ones-cols): vA[tok, h*65:h*65+65] = [x @ Wv_h.T (+bv) | 1]
                    mt = tok_sizes[i]
                    pvt = pv_pool.tile([128, 260], f32, tag="pv")
                    for e in range(8):
                        nc.tensor.matmul(
                            pvt[0:mt, :],
                            xT[:, e, i * 128:i * 128 + mt],
                            wv_sb[:, f, e, :],
                            start=(e == 0), stop=(e == 7 and not with_bias),
                        )
                    if with_bias:
                        nc.tensor.matmul(
                            pvt[0:mt, :], ones[0:1, 0:mt],
                            bv_sb[0:1, f * 260:(f + 1) * 260],
                            start=False, stop=True,
                        )
                    nc.vector.tensor_copy(
                        vA[0:mt, i, f * 260:(f + 1) * 260], pvt[0:mt, :]
                    )
                    if not with_bias:
                        # ones columns via strided add (psum zeros there)
                        ov_view = vA[0:mt, i, f * 260:(f + 1) * 260].rearrange(
                            "p (h c) -> p h c", c=65
                        )[:, :, 64:65]
                        nc.gpsimd.tensor_scalar_add(ov_view, ov_view, 1.0)

                def vtail_dma(b):
                    # replicate the per-block tail-ctx V rows across all four
                    # 32-partition groups so packed-tail PV matmuls line up
                    for r in range(4):
                        nc.sync.dma_start(
                            vTail[r * 32:(r + 1) * 32, b, :], vA[0:32, 2 * b + 2, :]
                        )

                def emit_recip_chain(b):
                    # gather this block's 16 sum rows from the DRAM scratch,
                    # reciprocate once on DVE, and bounce back through DRAM
                    # into the paired layout (row 0 = even heads, row 1 = odd)
                    nc.sync.dma_start(
                        s_half[b][:],
                        d_ssc[b][0:1, :].rearrange("p (a f) -> (p a) f", a=16),
                    )
                    nc.vector.reciprocal(r_half[b][:], s_half[b][:])
                    nc.sync.dma_start(d_rsc[b][:], r_half[b][:])
                    nc.sync.dma_start(
                        r2[0:2, b * 8 * TB:(b + 1) * 8 * TB].rearrange(
                            "p (a f) -> p a f", f=TB
                        ),
                        d_rsc[b][:].rearrange("(a p) f -> p a f", p=2),
                    )

                def norm_unit(b, c):
                    # rank-2 selector broadcast: one matmul + one multiply
                    # normalizes a head pair of oT
                    rb2 = pv_pool.tile([128, TB], f32, tag="pv")
                    nc.tensor.matmul(
                        rb2[:], sel2[:],
                        r2[0:2, (b * 8 + c) * TB:(b * 8 + c + 1) * TB]
                        .bitcast(f32r),
                        start=True, stop=True,
                    )
                    sl = oT[:, c, b * TB:(b + 1) * TB]
                    nc.vector.tensor_tensor(sl, sl, rb2[:], MUL)

                def out_unit(b, m, f):
                    # out-projection y[tok, :] = oT.T @ wp (+ bp)
                    pa3 = pov.tile([128, 512], f32, tag="ov")
                    for c in range(8):
                        nc.tensor.matmul(
                            pa3[:], oT[:, c, m * 128:(m + 1) * 128],
                            wp_sb[:, f, c, :], start=(c == 0),
                            stop=(c == 7 and not with_bias),
                        )
                    if with_bias:
                        nc.tensor.matmul(
                            pa3[:], ones[0:1, 0:128],
                            bp_sb[0:1, f * 512:(f + 1) * 512],
                            start=False, stop=True,
                        )
                    ot = out_pool.tile([128, 512], f32)
                    nc.scalar.copy(ot[:], pa3[:])
                    nc.sync.dma_start(
                        d_y[m * 128:(m + 1) * 128, f * 512:(f + 1) * 512], ot[:]
                    )

                def emit_qk(b, g):
                    base = b * TB
                    # QK bursts kind-by-kind so each LDWEIGHTS prefetches
                    # under the previous head's matmul
                    lgs = []
                    for hh in range(4):
                        ro = hh * 32
                        qh = qT[ro:ro + 32, g, base:base + TB]
                        lg = plgM.tile([128, 512], f32)
                        nc.tensor.matmul(
                            lg[:, 0:256], kT[ro:ro + 32, g, base:base + 128],
                            qh, start=True, stop=True, tile_position=(ro, 0),
                        )
                        lgs.append(lg)
                    exMs = []
                    for hh in range(4):
                        ro = hh * 32
                        qh = qT[ro:ro + 32, g, base:base + TB]
                        nc.tensor.matmul(
                            lgs[hh][:, 256:512],
                            kT[ro:ro + 32, g, base + 128:base + 256],
                            qh, start=True, stop=True, tile_position=(ro, 0),
                        )
                        nc.vector.tensor_tensor(
                            lgs[hh][:], lgs[hh][:], mask_sb[:, b, 0:512], ADD
                        )
                        ex = exp_pool.tile([128, 512], bf16)
                        nc.scalar.activation(ex[:], lgs[hh][:], EXP)
                        exMs.append(ex)
                    lgt = plgT.tile([128, 512], f32)
                    for hh in range(4):
                        ro = hh * 32
                        qh = qT[ro:ro + 32, g, base:base + TB]
                        nc.tensor.matmul(
                            lgt[ro:ro + 32, 0:256],
                            kT[ro:ro + 32, g, base + 256:base + 288],
                            qh, start=True, stop=True, tile_position=(ro, ro),
                        )
                        nc.tensor.matmul(
                            lgt[ro:ro + 32, 256:512],
                            kbuf_sb[ro:ro + 32, g, :],
                            qh, start=True, stop=True, tile_position=(ro, ro),
                        )
                    nc.vector.tensor_tensor(
                        lgt[:], lgt[:], mask_sb[:, b, 512:1024], ADD
                    )
                    exT = exp_pool.tile([128, 512], bf16)
                    nc.scalar.activation(exT[:], lgt[:], EXP)
                    return exMs, exT

                def emit_pv(b, g, exMs, exT):
                    base = b * TB
                    cur_ov = None
                    for hh in range(4):
                        h = g * 4 + hh
                        p2, q = divmod(hh, 2)
                        hc = h * 65
                        if q == 0:
                            cur_ov = pov.tile([128, 512], f32, tag="ov")
                        ovr = cur_ov[:, q * 256:q * 256 + 256]
                        nc.tensor.matmul(
                            ovr[0:65, :], vA[:, 2 * b, hc:hc + 65],
                            exMs[hh][:, 0:256], start=True, stop=False,
                        )
                        nc.tensor.matmul(
                            ovr[0:65, :], vA[:, 2 * b + 1, hc:hc + 65],
                            exMs[hh][:, 256:512], start=False, stop=False,
                        )
                        ro = hh * 32
                        nc.tensor.matmul(
                            ovr[0:65, :], vTail[ro:ro + 32, b, hc:hc + 65],
                            exT[ro:ro + 32, 0:256], start=False, stop=False,
                            tile_position=(ro, 0),
                        )
                        nc.tensor.matmul(
                            ovr[0:65, :], vbuf_sb[ro:ro + 32, hc:hc + 65],
                            exT[ro:ro + 32, 256:512], start=False, stop=True,
                            tile_position=(ro, 0),
                        )
                        od = oT[(h % 2) * 64:(h % 2) * 64 + 64, h // 2,
                                base:base + TB]
                        if hh < 2:
                            nc.scalar.copy(od, ovr[0:64, :])
                        else:
                            nc.vector.tensor_copy(od, ovr[0:64, :])
                        if q == 1:
                            sst = s_pool.tile([1, 2 * TB], f32)
                            if hh < 2:
                                nc.vector.tensor_copy(sst[:], cur_ov[64:65, :])
                            else:
                                nc.scalar.copy(sst[:], cur_ov[64:65, :])
                            lidx = h - 1 - 0  # head-pair row within the block
                            nc.sync.dma_start(
                                d_ssc[b][0:1, lidx * TB:(lidx + 2) * TB],
                                sst[:],
                            )

                # interleave schedule: fillers[it] = (pre-PV units, post-PV units)
                fillers = {
                    -1: [lambda: v_unit(0, 0), lambda: v_unit(0, 1),
                         lambda: v_unit(0, 2), lambda: v_unit(1, 2),
                         lambda: v_unit(2, 2), lambda: v_unit(3, 2),
                         lambda: vtail_dma(0)],
                    0: [lambda: v_unit(1, 0), lambda: v_unit(1, 1)],
                    1: [lambda: v_unit(2, 0), lambda: v_unit(2, 1)],
                    2: [lambda: v_unit(3, 0), lambda: v_unit(3, 1)],
                    3: [lambda: v_unit(0, 3), lambda: v_unit(0, 4)],
                    4: [lambda: v_unit(1, 4), lambda: v_unit(2, 4),
                        lambda: v_unit(3, 4), lambda: vtail_dma(1),
                        lambda: v_unit(1, 3)],
                    5: [lambda: v_unit(2, 3), lambda: v_unit(3, 3),
                        lambda: norm_unit(0, 0), lambda: norm_unit(0, 1)],
                    6: [lambda: norm_unit(0, 2), lambda: norm_unit(0, 3),
                        lambda: norm_unit(0, 4), lambda: norm_unit(0, 5),
                        lambda: norm_unit(0, 6), lambda: norm_unit(0, 7),
                        lambda: out_unit(0, 0, 0)],
                    7: [lambda: out_unit(0, 0, 1), lambda: out_unit(0, 1, 0)],
                    8: [lambda: out_unit(0, 1, 1)],
                }

                for f in fillers[-1]:
                    f()
                prev = None
                for it in range(NB * 4 + 1):
                    if it < NB * 4:
                        b, g = divmod(it, 4)
                        exMs, exT = emit_qk(b, g)
                        cur = (b, g, exMs, exT)
                    else:
                        cur = None
                    fl = fillers.get(it, [])
                    mid = max(0, len(fl) - 2)
                    for f in fl[:mid]:
                        f()
                    if prev is not None:
                        emit_pv(*prev)
                    for f in fl[mid:]:
                        f()
                    prev = cur
                    if it == 4:
                        emit_recip_chain(0)

                # tail: block-1 normalization + output projection
                emit_recip_chain(1)
                for c in range(8):
                    norm_unit(1, c)
                out_unit(1, 2, 0)
                out_unit(1, 2, 1)
                out_unit(1, 3, 0)
                out_unit(1, 3, 1)
    return nc


def _get_runner(with_bias=True):
    key = ("runner", with_bias)
    if key in _CACHE:
        return _CACHE[key]
    import jax
    import concourse.mybir as mybir
    from concourse import bass2jax
    from jax.sharding import Mesh, PartitionSpec
    from jax.experimental.shard_map import shard_map

    nc = _build_nc(with_bias)
    bass2jax.install_neuronx_cc_hook()
    partition_name = nc.partition_id_tensor.name if nc.partition_id_tensor else None
    in_names, out_names, out_avals, out_shapes = [], [], [], []
    for alloc in nc.m.functions[0].allocations:
        if not isinstance(alloc, mybir.MemoryLocationSet):
            continue
        name = alloc.memorylocations[0].name
        if alloc.kind == "ExternalInput":
            if name != partition_name:
                in_names.append(name)
        elif alloc.kind == "ExternalOutput":
            shape = tuple(alloc.tensor_shape)
            dtype = mybir.dt.np(alloc.dtype)
            out_names.append(name)
            out_avals.append(jax.core.ShapedArray(shape, dtype))
            out_shapes.append((shape, dtype))
    n_params = len(in_names)
    n_outs = len(out_avals)
    all_in_names = in_names + out_names + ([partition_name] if partition_name else [])
    donate = tuple(range(n_params, n_params + n_outs))

    def _body(*args):
        operands = list(args)
        if partition_name is not None:
            operands.append(bass2jax.partition_id_tensor())
        outs = bass2jax._bass_exec_p.bind(
            *operands,
            out_avals=tuple(out_avals),
            in_names=tuple(all_in_names),
            out_names=tuple(out_names),
            lowering_input_output_aliases=(),
            sim_require_finite=True,
            sim_require_nnan=True,
            nc=nc,
        )
        return tuple(outs)

    devices = jax.devices()[:N_CORES]
    mesh = Mesh(np.asarray(devices), ("core",))
    sharded = jax.jit(
        shard_map(
            _body, mesh=mesh,
            in_specs=(PartitionSpec("core"),) * (n_params + n_outs),
            out_specs=(PartitionSpec("core"),) * n_outs,
            check_rep=False,
        ),
        donate_argnums=donate,
        keep_unused=True,
    )

    def run(in_maps):
        per_core = [[np.asarray(m[name]) for name in in_names] for m in in_maps]
        concat_in = [
            np.concatenate([per_core[c][i] for c in range(N_CORES)], axis=0)
            for i in range(n_params)
        ]
        concat_zeros = [
            np.zeros((N_CORES * s[0], *s[1:]), d) for (s, d) in out_shapes
        ]
        out_arrs = sharded(*concat_in, *concat_zeros)
        return [
            {
                name: np.asarray(out_arrs[i]).reshape(N_CORES, *out_shapes[i][0])[c]
                for i, name in enumerate(out_names)
            }
            for c in range(N_CORES)
        ]

    _CACHE[key] = run
    return run


# ------------------------------------------------------------------- host
def _prep_inputs(x, Wkv, bkv, Wq, bq, Wp, bp, buffer, sample_lengths):
    import ml_dtypes

    bfl = ml_dtypes.bfloat16
    x = np.asarray(x, np.float32)
    Wkv = np.asarray(Wkv, np.float32)
    bkv = np.asarray(bkv, np.float32)
    Wq = np.asarray(Wq, np.float32)
    bq = np.asarray(bq, np.float32)
    Wp = np.asarray(Wp, np.float32)
    bp = np.asarray(bp, np.float32)
    buffer = np.asarray(buffer, np.float32)
    lengths = np.asarray(sample_lengths).astype(np.int64)

    scale = 1.0 / math.sqrt(DK)
    starts = np.concatenate([[0], np.cumsum(lengths)[:-1]]).astype(np.int64)
    t = np.arange(T)
    seg = np.searchsorted(starts, t, side="right") - 1
    j = t - starts[seg]

    # weights pre-rearranged into exact SBUF layouts ([p, ...] partition-major)
    wkT = np.ascontiguousarray(Wkv[:KD, :].T)                       # [E, KD]
    wk_h = wkT.reshape(8, 128, 4, 128).transpose(1, 0, 2, 3).astype(bfl)
    wqT = np.ascontiguousarray(Wq.T * scale)                        # [E, KD]
    wq_h = wqT.reshape(8, 128, 4, 128).transpose(1, 0, 2, 3).astype(bfl)
    wv_aug = np.zeros((E, H, 65), np.float32)
    wv_aug[:, :, :64] = Wkv[KD:, :].T.reshape(E, H, DV)
    wv_h = (
        wv_aug.reshape(E, H * 65).reshape(8, 128, 4, 260)
        .transpose(1, 2, 0, 3).astype(bfl)
    )
    wpT = np.ascontiguousarray(Wp.T)                                # [E, E]
    wp_h = wpT.reshape(8, 128, 2, 512).transpose(1, 2, 0, 3).astype(bfl)

    bk2 = np.ascontiguousarray(bkv[:KD].reshape(4, 128).T)
    bq2 = np.ascontiguousarray((bq * scale).reshape(4, 128).T)
    bv_aug = np.zeros((H, 65), np.float32)
    bv_aug[:, :64] = bkv[KD:].reshape(H, DV)
    bv_aug[:, 64] = 1.0
    bv_row = np.ascontiguousarray(bv_aug.reshape(1, H * 65))
    bp_row = np.ascontiguousarray(bp[None, :])
    ones_row = np.ones((1, 128), np.float32)
    sel2 = np.zeros((2, 128), np.float32)
    sel2[0, :64] = 1.0
    sel2[1, 64:] = 1.0

    kbufT = np.zeros((KD, 32), np.float32)
    kbufT[:, :HALO] = buffer[:, :KD].T
    kbuf_h = kbufT.reshape(4, 128, 32).transpose(1, 0, 2).astype(bfl)
    vbuf = np.zeros((32, H * 65), np.float32)
    vb = vbuf.reshape(32, H, 65)
    vb[:HALO, :, :64] = buffer[:, KD:].reshape(HALO, H, DV)
    vb[:HALO, :, 64] = 1.0
    vbuf_h = vbuf.astype(bfl)

    xTp = np.zeros((E, T + HALO + 33), np.float32)
    xTp[:, HALO:HALO + T] = x.T

    in_maps = []
    for c in range(N_CORES):
        t0 = c * SHARD
        xT_c = np.ascontiguousarray(
            xTp[:, t0:t0 + NTOK].reshape(8, 128, NTOK).transpose(1, 0, 2)
        ).astype(bfl)
        mask = np.full((128, NB, 1024), NEG, np.float32)
        for bblk in range(NB):
            i = np.arange(TB)
            tt = t0 + bblk * TB + i
            st = starts[seg[tt]]
            jj = j[tt]
            for r in range(2):
                p = np.arange(128)[:, None]
                g = t0 - HALO + bblk * TB + r * 128 + p
                valid = (
                    (g >= tt[None, :] - HALO) & (g <= tt[None, :])
                    & (g >= st[None, :]) & (g >= 0) & (g < T)
                )
                mask[:, bblk, r * 256:(r + 1) * 256] = np.where(valid, 0.0, NEG)
            p32 = np.arange(32)[:, None]
            g = t0 - HALO + bblk * TB + 256 + p32
            valid = (
                (g >= tt[None, :] - HALO) & (g <= tt[None, :])
                & (g >= st[None, :]) & (g >= 0) & (g < T)
            )
            tailm = np.where(valid, 0.0, NEG)
            pb = np.arange(32)[:, None]
            validb = (pb >= jj[None, :]) & (pb <= HALO - 1)
            bufm = np.where(validb, 0.0, NEG)
            for rr in range(4):
                mask[rr * 32:(rr + 1) * 32, bblk, 512:768] = tailm
                mask[rr * 32:(rr + 1) * 32, bblk, 768:1024] = bufm
        in_maps.append({
            "xT": xT_c, "wk": wk_h, "wq": wq_h, "wv": wv_h, "wp": wp_h,
            "kbufT": kbuf_h, "vbuf": vbuf_h, "ones": ones_row, "sel2": sel2,
            "mask": np.ascontiguousarray(mask.astype(bfl)),
            "bk": bk2, "bq": bq2, "bv": bv_row, "bp": bp_row,
        })
    return in_maps, seg, j


def kernel(x, Wkv, bkv, Wq, bq, Wp, bp, buffer, sample_lengths):
    in_maps, seg, j = _prep_inputs(
        x, Wkv, bkv, Wq, bq, Wp, bp, buffer, sample_lengths
    )
    with_bias = bool(
        np.any(np.asarray(bkv)) or np.any(np.asarray(bq)) or np.any(np.asarray(bp))
    )
    run = _get_runner(with_bias)
    results = run(in_maps)
    out_full = np.concatenate([results[c]["yout"] for c in range(N_CORES)], axis=0)
    y = np.zeros((B, MAXL, E), np.float32)
    ok = j < MAXL
    y[seg[ok], j[ok]] = out_full[ok]
    return y


# revision 24
# speedup vs baseline: 1.1001x; 1.1001x over previous
"""Trainium2 Bass kernel for nn_DistiledMultiheadAttention_76476187673064.

Sliding-window (W=32) single-query attention over ragged sequences with a
learned pre-context buffer, plus input/output projections.

Strategy (8 NeuronCores, data-parallel over flat tokens):
  - Each core owns 512 tokens; kv for a 31-token halo is recomputed locally
    (plus one masked pad column), so no collectives are needed.
  - All matmul operands are bf16 (host-cast, fp32 PSUM accumulation).
  - HAM-warmth-driven schedule: the attention QK/PV matmuls have low PE
    array activity (32/128 contraction rows resp. 65/128 output columns),
    which keeps the PE clock-gated at 1.2GHz if they run as a contiguous
    phase.  This version interleaves the dense full-array work (V
    projection, output projection, normalization) INTO the attention
    iterations so every HAM activity window sees high utilization and the
    PE stays at 2.4GHz.
  - Startup: x and Wk are split into per-e-chunk DMAs issued concurrently
    from the Sync AND Activation HWDGE queues; K projection runs e-outer
    (4 PSUM accumulators live) so matmuls start as soon as the first
    chunks land instead of after the full tensors.
  - Additive band/segment/buffer masks are host-precomputed in bf16
    (half the HBM traffic); exp without max-subtraction (logits bounded).
  - ScalarE runs ONLY Exp + Copy (single ACT table load); softmax sums
    are gathered per block and reciprocated by one DVE InstReciprocal;
    normalization applied as rank-2 broadcast matmuls into the attention
    output ahead of the output projection.
"""
import math
import sys

sys.path.insert(0, "/opt/trn_rl_repo")

import numpy as np

# ---------------------------------------------------------------- constants
T = 4096
E = 1024
KD = 512          # key dim
H = 16            # heads
W = 32            # window
DK = KD // H      # 32
DV = E // H       # 64
B = 8
MAXL = 768
N_CORES = 8
SHARD = T // N_CORES          # 512 tokens per core
HALO = W - 1                  # 31
NTOK = SHARD + HALO + 1       # 544 token columns incl. halo + 1 pad
TB = 256                      # attention token block
NB = SHARD // TB              # 2 blocks per core
NEG = -30000.0

_CACHE = {}


# ------------------------------------------------------------- tile patches
def _apply_tile_patches():
    """This container's walrus only supports ONE sync-wait per instruction;
    redistribute extra Tile-assigned waits onto single-wait InstNoOp carriers."""
    import concourse.mybir as mybir
    import concourse.tile as tile
    from concourse.vector_clock import ScopedClock

    if getattr(tile.TileContext, "_wait_split_patched", False):
        return
    orig_commit = tile.TileContext._commit_and_lower

    def commit_split(self, inst, original_block, old_bb_map, bb_to_exit_bb):
        si = getattr(inst, "sync_info", None)
        if si is not None and si.on_wait and len(si.on_wait) > 1:
            engine = inst.engine
            if engine is not None and engine != mybir.EngineType.Unassigned:
                waits = list(si.on_wait)
                si.on_wait = waits[-1:]
                for w in waits[:-1]:
                    noop = mybir.InstNoOp(
                        name=self.nc.get_next_instruction_name(),
                        sync_info=mybir.SyncInfo(on_wait=[w], on_update=[]),
                        bass_nofuse=True,
                        engine=engine,
                        text_hint="wait_split",
                    )
                    orig_commit(self, noop, original_block, old_bb_map, bb_to_exit_bb)
        return orig_commit(self, inst, original_block, old_bb_map, bb_to_exit_bb)

    def drain_and_barrier(self, tick_clock, wait_clock):
        drain_inst = self.nc.sync.drain()
        wait_clock.add_sem_waits(
            drain_inst.ins, ScopedClock({None: tick_clock.global_clock})
        )
        si = drain_inst.ins.sync_info
        if si is not None and si.on_wait and len(si.on_wait) > 1:
            waits = list(si.on_wait)
            si.on_wait = waits[:1]
            for w in waits[1:]:
                nop = self.nc.sync.nop(nofuse=True)
                nsi = nop.ins.sync_info
                if nsi is None:
                    nop.ins.sync_info = mybir.SyncInfo(on_wait=[w], on_update=[])
                else:
                    nsi.on_wait = list(nsi.on_wait or []) + [w]
        self.nc.all_engine_barrier()
        assert self.sems is not None
        popped = self.nc._tile_sem_poison_stack.pop()
        assert popped is self._sem_poison
        self.nc.clear_and_free_semaphores(list(self.sems.allocated().values()))
        self.nc.all_engine_barrier()

    tile.TileContext._commit_and_lower = commit_split
    tile.TileContext._drain_and_barrier = drain_and_barrier
    tile.TileContext._wait_split_patched = True


# ------------------------------------------------------------- device build
def _build_nc(with_bias=True):
    import concourse.bass as bass
    import concourse.mybir as mybir
    import concourse.tile as tile

    _apply_tile_patches()
    f32 = mybir.dt.float32
    f32r = mybir.dt.float32r
    bf16 = mybir.dt.bfloat16
    ADD = mybir.AluOpType.add
    MUL = mybir.AluOpType.mult
    EXP = mybir.ActivationFunctionType.Exp

    nc = bass.Bass()
    d_xT = nc.dram_tensor("xT", [128, 8, NTOK], bf16, kind="ExternalInput")
    d_wk = nc.dram_tensor("wk", [128, 8, 4, 128], bf16, kind="ExternalInput")
    d_wq = nc.dram_tensor("wq", [128, 8, 4, 128], bf16, kind="ExternalInput")
    d_wv = nc.dram_tensor("wv", [128, 4, 8, 260], bf16, kind="ExternalInput")
    d_wp = nc.dram_tensor("wp", [128, 2, 8, 512], bf16, kind="ExternalInput")
    d_kbufT = nc.dram_tensor("kbufT", [128, 4, 32], bf16, kind="ExternalInput")
    d_vbuf = nc.dram_tensor("vbuf", [32, 1040], bf16, kind="ExternalInput")
    d_ones = nc.dram_tensor("ones", [1, 128], f32r, kind="ExternalInput")
    d_mask = nc.dram_tensor("mask", [128, NB, 1024], f32, kind="ExternalInput")
    d_bk = nc.dram_tensor("bk", [128, 4], f32, kind="ExternalInput")
    d_bq = nc.dram_tensor("bq", [128, 4], f32, kind="ExternalInput")
    d_bv = nc.dram_tensor("bv", [1, H * 65], f32r, kind="ExternalInput")
    d_bp = nc.dram_tensor("bp", [1, E], f32r, kind="ExternalInput")
    d_y = nc.dram_tensor("yout", [SHARD, E], f32, kind="ExternalOutput")
    # sums/recips bounce through DRAM to move data across partitions
    d_rsc = [nc.dram_tensor(f"rscratch{i}", [128, 32], f32, kind="Internal")
             for i in range(NB)]
    d_ssc = [nc.dram_tensor(f"sscratch{i}", [1, 16 * TB], f32, kind="Internal")
             for i in range(NB)]

    with tile.TileContext(nc) as tc, nc.allow_low_precision(
        reason="bf16 matmul operands; fp32 PSUM accumulation throughout"
    ):
        with (
            tc.tile_pool(name="x", bufs=1) as x_pool,
            tc.tile_pool(name="wgt", bufs=1) as w_pool,
            tc.tile_pool(name="const", bufs=1) as const_pool,
            tc.tile_pool(name="kqv", bufs=1) as kqv_pool,
            tc.tile_pool(name="exp", bufs=12) as exp_pool,
            tc.tile_pool(name="srow", bufs=4) as s_pool,
            tc.tile_pool(name="nbc", bufs=3) as nb_pool,
            tc.tile_pool(name="out", bufs=3) as out_pool,
        ):
            # ---- persistent SBUF tiles
            xT = x_pool.tile([128, 8, NTOK], bf16)
            wk_sb = w_pool.tile([128, 8, 4, 128], bf16)
            wq_sb = w_pool.tile([128, 8, 4, 128], bf16)
            wv_sb = w_pool.tile([128, 4, 8, 260], bf16)
            wp_sb = w_pool.tile([128, 2, 8, 512], bf16)
            kbuf_sb = const_pool.tile([128, 4, 32], bf16)
            vbuf_sb = const_pool.tile([128, 1040], bf16)
            ones = const_pool.tile([1, 128], f32r)
            mask_sb = const_pool.tile([128, NB, 1024], f32)
            if with_bias:
                bk_sb = const_pool.tile([128, 4], f32)
                bq_sb = const_pool.tile([128, 4], f32)
                bv_sb = const_pool.tile([1, H * 65], f32r)
                bp_sb = const_pool.tile([1, E], f32r)

            # ---- input DMAs, split + spread across both HWDGE engines so
            # K projection can start as soon as the first chunks land.
            # sync: x chunks + everything attention needs early
            # scalar: wk chunks, wv, then late consumers (mask b1, wp)
            for e in range(8):
                nc.sync.dma_start(xT[:, e, :], d_xT[:, e, :])
                nc.scalar.dma_start(wk_sb[:, e, :, :], d_wk[:, e, :, :])
            nc.sync.dma_start(wq_sb[:], d_wq[:])
            nc.scalar.dma_start(wv_sb[:], d_wv[:])
            nc.sync.dma_start(kbuf_sb[:], d_kbufT[:])
            for r in range(4):
                nc.sync.dma_start(vbuf_sb[r * 32:(r + 1) * 32, :], d_vbuf[:])
            nc.sync.dma_start(ones[:], d_ones[:])
            nc.sync.dma_start(mask_sb[:, 0, :], d_mask[:, 0, :])
            if with_bias:
                nc.sync.dma_start(bk_sb[:], d_bk[:])
                nc.sync.dma_start(bq_sb[:], d_bq[:])
                nc.sync.dma_start(bv_sb[:], d_bv[:])
                nc.sync.dma_start(bp_sb[:], d_bp[:])
            nc.scalar.dma_start(mask_sb[:, 1, :], d_mask[:, 1, :])
            nc.scalar.dma_start(wp_sb[:], d_wp[:])

            # ---- persistent activations
            kT = kqv_pool.tile([128, 4, NTOK], bf16)    # K feature-major
            qT = kqv_pool.tile([128, 4, SHARD], bf16)   # Q feature-major (scaled)
            vA = kqv_pool.tile([128, 5, H * 65], bf16)  # V token-major + ones col
            vTail = kqv_pool.tile([128, NB, H * 65], bf16)  # tail V, 4x replicated
            oT = kqv_pool.tile([128, 8, SHARD], bf16)   # attention out feature-major
            # sums/recips: partition p = pair*16 + parity*8 + tc, col = tok%32
            s_half = [kqv_pool.tile([128, 32], f32, name=f"s_half{i}")
                      for i in range(NB)]
            r_half = [kqv_pool.tile([128, 32], f32, name=f"r_half{i}")
                      for i in range(NB)]
            # recips broadcast across partitions (rows 0:64 = even head of
            # the pair, 64:128 = odd), one [128, TB] slab per head pair
            bc_all = [kqv_pool.tile([128, 8, TB], f32, name=f"bc_all{i}")
                      for i in range(NB)]

            # ================= K/Q projection (e-outer so compute starts
            # as soon as chunk 0 of x and Wk arrive)
            with (
                tc.tile_pool(name="ppa", bufs=4, space="PSUM") as ppa,
                tc.tile_pool(name="ppb", bufs=4, space="PSUM") as ppb,
            ):
                pa = [ppa.tile([128, 512], f32, tag="pa", name=f"pa{m}")
                      for m in range(4)]
                pbt = [ppb.tile([128, 32], f32, tag="pb", name=f"pb{m}")
                       for m in range(4)]
                for e in range(8):
                    for m in range(4):
                        nc.tensor.matmul(
                            pa[m][:], wk_sb[:, e, m, :], xT[:, e, 0:512],
                            start=(e == 0), stop=(e == 7),
                        )
                        nc.tensor.matmul(
                            pbt[m][:], wk_sb[:, e, m, :], xT[:, e, 512:NTOK],
                            start=(e == 0), stop=(e == 7),
                        )
                for m in range(4):
                    if with_bias:
                        nc.scalar.add(kT[:, m, 0:512], pa[m][:], bk_sb[:, m:m + 1])
                        nc.scalar.add(kT[:, m, 512:NTOK], pbt[m][:],
                                      bk_sb[:, m:m + 1])
                    elif m % 2 == 0:
                        nc.scalar.copy(kT[:, m, 0:512], pa[m][:])
                        nc.scalar.copy(kT[:, m, 512:NTOK], pbt[m][:])
                    else:
                        nc.vector.tensor_copy(kT[:, m, 0:512], pa[m][:])
                        nc.vector.tensor_copy(kT[:, m, 512:NTOK], pbt[m][:])

                # Q projection (tokens only, no halo)
                qa = [ppa.tile([128, 512], f32, tag="pa", name=f"qa{m}")
                      for m in range(4)]
                for e in range(8):
                    for m in range(4):
                        nc.tensor.matmul(
                            qa[m][:], wq_sb[:, e, m, :], xT[:, e, HALO:HALO + SHARD],
                            start=(e == 0), stop=(e == 7),
                        )
                for m in range(4):
                    if with_bias:
                        nc.scalar.add(qT[:, m, :], qa[m][:], bq_sb[:, m:m + 1])
                    elif m % 2 == 0:
                        nc.scalar.copy(qT[:, m, :], qa[m][:])
                    else:
                        nc.vector.tensor_copy(qT[:, m, :], qa[m][:])

            # ================= attention + V projection + output projection,
            # interleaved so the PE array activity stays high (HAM warm)
            with (
                tc.tile_pool(name="pv", bufs=1, space="PSUM") as pv_pool,
                tc.tile_pool(name="plgM", bufs=4, space="PSUM") as plgM,
                tc.tile_pool(name="plgT", bufs=1, space="PSUM") as plgT,
                tc.tile_pool(name="pov", bufs=2, space="PSUM") as pov,
            ):
                tok_sizes = [128, 128, 128, 128, 32]

                def v_unit(f, i):
                    # V projection token-major (wv pre-augmented with zero
                    # ones-cols): vA[tok, h*65:h*65+65] = [x @ Wv_h.T (+bv) | 1]
                    mt = tok_sizes[i]
                    pvt = pv_pool.tile([128, 260], f32, tag="pv")
                    for e in range(8):
                        nc.tensor.matmul(
                            pvt[0:mt, :],
                            xT[:, e, i * 128:i * 128 + mt],
                            wv_sb[:, f, e, :],
                            start=(e == 0), stop=(e == 7 and not with_bias),
                        )
                    if with_bias:
                        nc.tensor.matmul(
                            pvt[0:mt, :], ones[0:1, 0:mt],
                            bv_sb[0:1, f * 260:(f + 1) * 260],
                            start=False, stop=True,
                        )
                    nc.vector.tensor_copy(
                        vA[0:mt, i, f * 260:(f + 1) * 260], pvt[0:mt, :]
                    )
                    if not with_bias:
                        # ones columns via strided add (psum zeros there)
                        ov_view = vA[0:mt, i, f * 260:(f + 1) * 260].rearrange(
                            "p (h c) -> p h c", c=65
                        )[:, :, 64:65]
                        nc.gpsimd.tensor_scalar_add(ov_view, ov_view, 1.0)

                def vtail_dma(b):
                    # replicate the per-block tail-ctx V rows across all four
                    # 32-partition groups so packed-tail PV matmuls line up
                    for r in range(4):
                        nc.sync.dma_start(
                            vTail[r * 32:(r + 1) * 32, b, :], vA[0:32, 2 * b + 2, :]
                        )

                def emit_chain(b, half):
                    # reciprocate one half-block of softmax sums (4 head
                    # pairs, 64 partitions), bounce through DRAM, and read
                    # back broadcast across partitions into bc_all
                    lo = half * 64
                    oc = half * 4
                    nc.sync.dma_start(
                        s_half[b][lo:lo + 64, :],
                        d_ssc[b][0:1, half * 2048:half * 2048 + 2048]
                        .rearrange("p (r j) -> (p r) j", j=32),
                    )
                    nc.vector.reciprocal(
                        r_half[b][lo:lo + 64, :], s_half[b][lo:lo + 64, :]
                    )
                    nc.sync.dma_start(
                        d_rsc[b][lo:lo + 64, :], r_half[b][lo:lo + 64, :]
                    )
                    src = d_rsc[b][lo:lo + 64, :].rearrange(
                        "(c q tc) j -> q c tc j", q=2, tc=8
                    )
                    for parity, prow in ((0, 0), (1, 64)):
                        nc.sync.dma_start(
                            bc_all[b][prow:prow + 64, oc:oc + 4, :].rearrange(
                                "p c (tc j) -> p c tc j", j=32
                            ),
                            src[parity:parity + 1].partition_broadcast(64),
                        )

                def norm_unit(b, c):
                    # single DVE multiply against the pre-broadcast recips
                    sl = oT[:, c, b * TB:(b + 1) * TB]
                    nc.vector.tensor_tensor(sl, sl, bc_all[b][:, c, :], MUL)

                def out_unit(b, m, f):
                    # out-projection y[tok, :] = oT.T @ wp (+ bp)
                    pa3 = pov.tile([128, 512], f32, tag="ov")
                    for c in range(8):
                        nc.tensor.matmul(
                            pa3[:], oT[:, c, m * 128:(m + 1) * 128],
                            wp_sb[:, f, c, :], start=(c == 0),
                            stop=(c == 7 and not with_bias),
                        )
                    if with_bias:
                        nc.tensor.matmul(
                            pa3[:], ones[0:1, 0:128],
                            bp_sb[0:1, f * 512:(f + 1) * 512],
                            start=False, stop=True,
                        )
                    ot = out_pool.tile([128, 512], f32)
                    nc.scalar.copy(ot[:], pa3[:])
                    nc.sync.dma_start(
                        d_y[m * 128:(m + 1) * 128, f * 512:(f + 1) * 512], ot[:]
                    )

                def emit_qk(b, g):
                    base = b * TB
                    # QK bursts kind-by-kind so each LDWEIGHTS prefetches
                    # under the previous head's matmul
                    lgs = []
                    for hh in range(4):
                        ro = hh * 32
                        qh = qT[ro:ro + 32, g, base:base + TB]
                        lg = plgM.tile([128, 512], f32)
                        nc.tensor.matmul(
                            lg[:, 0:256], kT[ro:ro + 32, g, base:base + 128],
                            qh, start=True, stop=True, tile_position=(ro, 0),
                        )
                        lgs.append(lg)
                    exMs = []
                    for hh in range(4):
                        ro = hh * 32
                        qh = qT[ro:ro + 32, g, base:base + TB]
                        nc.tensor.matmul(
                            lgs[hh][:, 256:512],
                            kT[ro:ro + 32, g, base + 128:base + 256],
                            qh, start=True, stop=True, tile_position=(ro, 0),
                        )
                        nc.vector.tensor_tensor(
                            lgs[hh][:], lgs[hh][:], mask_sb[:, b, 0:512], ADD
                        )
                        ex = exp_pool.tile([128, 512], bf16)
                        nc.scalar.activation(ex[:], lgs[hh][:], EXP)
                        exMs.append(ex)
                    lgt = plgT.tile([128, 512], f32)
                    for hh in range(4):
                        ro = hh * 32
                        qh = qT[ro:ro + 32, g, base:base + TB]
                        nc.tensor.matmul(
                            lgt[ro:ro + 32, 0:256],
                            kT[ro:ro + 32, g, base + 256:base + 288],
                            qh, start=True, stop=True, tile_position=(ro, ro),
                        )
                        nc.tensor.matmul(
                            lgt[ro:ro + 32, 256:512],
                            kbuf_sb[ro:ro + 32, g, :],
                            qh, start=True, stop=True, tile_position=(ro, ro),
                        )
                    nc.vector.tensor_tensor(
                        lgt[:], lgt[:], mask_sb[:, b, 512:1024], ADD
                    )
                    exT = exp_pool.tile([128, 512], bf16)
                    nc.scalar.activation(exT[:], lgt[:], EXP)
                    return exMs, exT

                def emit_pv(b, g, exMs, exT):
                    base = b * TB
                    cur_ov = None
                    for hh in range(4):
                        h = g * 4 + hh
                        p2, q = divmod(hh, 2)
                        hc = h * 65
                        if q == 0:
                            cur_ov = pov.tile([128, 512], f32, tag="ov")
                        ovr = cur_ov[:, q * 256:q * 256 + 256]
                        nc.tensor.matmul(
                            ovr[0:65, :], vA[:, 2 * b, hc:hc + 65],
                            exMs[hh][:, 0:256], start=True, stop=False,
                        )
                        nc.tensor.matmul(
                            ovr[0:65, :], vA[:, 2 * b + 1, hc:hc + 65],
                            exMs[hh][:, 256:512], start=False, stop=False,
                        )
                        ro = hh * 32
                        nc.tensor.matmul(
                            ovr[0:65, :], vTail[ro:ro + 32, b, hc:hc + 65],
                            exT[ro:ro + 32, 0:256], start=False, stop=False,
                            tile_position=(ro, 0),
                        )
                        nc.tensor.matmul(
                            ovr[0:65, :], vbuf_sb[ro:ro + 32, hc:hc + 65],
                            exT[ro:ro + 32, 256:512], start=False, stop=True,
                            tile_position=(ro, 0),
                        )
                        od = oT[(h % 2) * 64:(h % 2) * 64 + 64, h // 2,
                                base:base + TB]
                        if hh < 2:
                            nc.scalar.copy(od, ovr[0:64, :])
                        else:
                            nc.vector.tensor_copy(od, ovr[0:64, :])
                        if q == 1:
                            sst = s_pool.tile([1, 2 * TB], f32)
                            if hh < 2:
                                nc.vector.tensor_copy(sst[:], cur_ov[64:65, :])
                            else:
                                nc.scalar.copy(sst[:], cur_ov[64:65, :])
                            pr = h // 2  # head-pair index within the block
                            nc.sync.dma_start(
                                d_ssc[b][0:1, pr * 512:(pr + 1) * 512], sst[:]
                            )

                # interleave schedule: pre[it] emitted between QK and PV,
                # post[it] emitted after PV — dense fillers keep HAM warm
                pre = {
                    0: [lambda: v_unit(1, 0), lambda: v_unit(1, 1)],
                    1: [lambda: v_unit(2, 0), lambda: v_unit(2, 1)],
                    2: [lambda: v_unit(3, 0), lambda: v_unit(3, 1)],
                    3: [lambda: v_unit(0, 3), lambda: v_unit(0, 4)],
                    4: [lambda: v_unit(1, 4), lambda: v_unit(2, 4),
                        lambda: v_unit(3, 4), lambda: vtail_dma(1)],
                    5: [lambda: v_unit(1, 3)],
                    6: [lambda: v_unit(2, 3)],
                    7: [lambda: v_unit(3, 3)],
                }
                post = {
                    2: [lambda: emit_chain(0, 0)],
                    3: [lambda: norm_unit(0, 0), lambda: norm_unit(0, 1)],
                    4: [lambda: emit_chain(0, 1),
                        lambda: norm_unit(0, 2), lambda: norm_unit(0, 3)],
                    5: [lambda: norm_unit(0, 4), lambda: norm_unit(0, 5),
                        lambda: norm_unit(0, 6), lambda: norm_unit(0, 7),
                        lambda: out_unit(0, 0, 0)],
                    6: [lambda: emit_chain(1, 0), lambda: out_unit(0, 0, 1)],
                    7: [lambda: norm_unit(1, 0), lambda: norm_unit(1, 1),
                        lambda: norm_unit(1, 2), lambda: norm_unit(1, 3),
                        lambda: out_unit(0, 1, 0)],
                }

                for f in [lambda: v_unit(0, 0), lambda: v_unit(0, 1),
                          lambda: v_unit(0, 2), lambda: v_unit(1, 2),
                          lambda: v_unit(2, 2), lambda: v_unit(3, 2),
                          lambda: vtail_dma(0)]:
                    f()
                prev = None
                for it in range(NB * 4 + 1):
                    if it < NB * 4:
                        b, g = divmod(it, 4)
                        exMs, exT = emit_qk(b, g)
                        cur = (b, g, exMs, exT)
                    else:
                        cur = None
                    for f in pre.get(it, []):
                        f()
                    if prev is not None:
                        emit_pv(*prev)
                    for f in post.get(it, []):
                        f()
                    prev = cur

                # tail: block-1 second recip half, remaining norms, and the
                # block-1 out-projection with its contraction loop split so
                # the first half runs while the last recips are in flight
                out_unit(0, 1, 1)
                ob = [plgM.tile([128, 512], f32, tag="lg", name=f"ob{u}")
                      for u in range(4)]
                ob_mf = [(2, 0), (2, 1), (3, 0), (3, 1)]
                for c in range(4):
                    for u, (m, f) in enumerate(ob_mf):
                        nc.tensor.matmul(
                            ob[u][:], oT[:, c, m * 128:(m + 1) * 128],
                            wp_sb[:, f, c, :], start=(c == 0), stop=False,
                        )
                emit_chain(1, 1)
                for c in range(4, 8):
                    norm_unit(1, c)
                for c in range(4, 8):
                    for u, (m, f) in enumerate(ob_mf):
                        nc.tensor.matmul(
                            ob[u][:], oT[:, c, m * 128:(m + 1) * 128],
                            wp_sb[:, f, c, :], start=False,
                            stop=(c == 7 and not with_bias),
                        )
                for u, (m, f) in enumerate(ob_mf):
                    if with_bias:
                        nc.tensor.matmul(
                            ob[u][:], ones[0:1, 0:128],
                            bp_sb[0:1, f * 512:(f + 1) * 512],
                            start=False, stop=True,
                        )
                    ot = out_pool.tile([128, 512], f32, name=f"obt{u}")
                    if u % 2 == 0:
                        nc.scalar.copy(ot[:], ob[u][:])
                    else:
                        nc.vector.tensor_copy(ot[:], ob[u][:])
                    nc.sync.dma_start(
                        d_y[m * 128:(m + 1) * 128, f * 512:(f + 1) * 512], ot[:]
                    )
    return nc


def _get_runner(with_bias=True):
    key = ("runner", with_bias)
    if key in _CACHE:
        return _CACHE[key]
    import jax
    import concourse.mybir as mybir
    from concourse import bass2jax
    from jax.sharding import Mesh, PartitionSpec
    from jax.experimental.shard_map import shard_map

    nc = _build_nc(with_bias)
    bass2jax.install_neuronx_cc_hook()
    partition_name = nc.partition_id_tensor.name if nc.partition_id_tensor else None
    in_names, out_names, out_avals, out_shapes = [], [], [], []
    for alloc in nc.m.functions[0].allocations:
        if not isinstance(alloc, mybir.MemoryLocationSet):
            continue
        name = alloc.memorylocations[0].name
        if alloc.kind == "ExternalInput":
            if name != partition_name:
                in_names.append(name)
        elif alloc.kind == "ExternalOutput":
            shape = tuple(alloc.tensor_shape)
            dtype = mybir.dt.np(alloc.dtype)
            out_names.append(name)
            out_avals.append(jax.core.ShapedArray(shape, dtype))
            out_shapes.append((shape, dtype))
    n_params = len(in_names)
    n_outs = len(out_avals)
    all_in_names = in_names + out_names + ([partition_name] if partition_name else [])
    donate = tuple(range(n_params, n_params + n_outs))

    def _body(*args):
        operands = list(args)
        if partition_name is not None:
            operands.append(bass2jax.partition_id_tensor())
        outs = bass2jax._bass_exec_p.bind(
            *operands,
            out_avals=tuple(out_avals),
            in_names=tuple(all_in_names),
            out_names=tuple(out_names),
            lowering_input_output_aliases=(),
            sim_require_finite=True,
            sim_require_nnan=True,
            nc=nc,
        )
        return tuple(outs)

    devices = jax.devices()[:N_CORES]
    mesh = Mesh(np.asarray(devices), ("core",))
    sharded = jax.jit(
        shard_map(
            _body, mesh=mesh,
            in_specs=(PartitionSpec("core"),) * (n_params + n_outs),
            out_specs=(PartitionSpec("core"),) * n_outs,
            check_rep=False,
        ),
        donate_argnums=donate,
        keep_unused=True,
    )

    def run(in_maps):
        per_core = [[np.asarray(m[name]) for name in in_names] for m in in_maps]
        concat_in = [
            np.concatenate([per_core[c][i] for c in range(N_CORES)], axis=0)
            for i in range(n_params)
        ]
        concat_zeros = [
            np.zeros((N_CORES * s[0], *s[1:]), d) for (s, d) in out_shapes
        ]
        out_arrs = sharded(*concat_in, *concat_zeros)
        return [
            {
                name: np.asarray(out_arrs[i]).reshape(N_CORES, *out_shapes[i][0])[c]
                for i, name in enumerate(out_names)
            }
            for c in range(N_CORES)
        ]

    _CACHE[key] = run
    return run


# ------------------------------------------------------------------- host
def _prep_inputs(x, Wkv, bkv, Wq, bq, Wp, bp, buffer, sample_lengths):
    import ml_dtypes

    bfl = ml_dtypes.bfloat16
    x = np.asarray(x, np.float32)
    Wkv = np.asarray(Wkv, np.float32)
    bkv = np.asarray(bkv, np.float32)
    Wq = np.asarray(Wq, np.float32)
    bq = np.asarray(bq, np.float32)
    Wp = np.asarray(Wp, np.float32)
    bp = np.asarray(bp, np.float32)
    buffer = np.asarray(buffer, np.float32)
    lengths = np.asarray(sample_lengths).astype(np.int64)

    scale = 1.0 / math.sqrt(DK)
    starts = np.concatenate([[0], np.cumsum(lengths)[:-1]]).astype(np.int64)
    t = np.arange(T)
    seg = np.searchsorted(starts, t, side="right") - 1
    j = t - starts[seg]

    # weights pre-rearranged into exact SBUF layouts ([p, ...] partition-major)
    wkT = np.ascontiguousarray(Wkv[:KD, :].T)                       # [E, KD]
    wk_h = wkT.reshape(8, 128, 4, 128).transpose(1, 0, 2, 3).astype(bfl)
    wqT = np.ascontiguousarray(Wq.T * scale)                        # [E, KD]
    wq_h = wqT.reshape(8, 128, 4, 128).transpose(1, 0, 2, 3).astype(bfl)
    wv_aug = np.zeros((E, H, 65), np.float32)
    wv_aug[:, :, :64] = Wkv[KD:, :].T.reshape(E, H, DV)
    wv_h = (
        wv_aug.reshape(E, H * 65).reshape(8, 128, 4, 260)
        .transpose(1, 2, 0, 3).astype(bfl)
    )
    wpT = np.ascontiguousarray(Wp.T)                                # [E, E]
    wp_h = wpT.reshape(8, 128, 2, 512).transpose(1, 2, 0, 3).astype(bfl)

    bk2 = np.ascontiguousarray(bkv[:KD].reshape(4, 128).T)
    bq2 = np.ascontiguousarray((bq * scale).reshape(4, 128).T)
    bv_aug = np.zeros((H, 65), np.float32)
    bv_aug[:, :64] = bkv[KD:].reshape(H, DV)
    bv_aug[:, 64] = 1.0
    bv_row = np.ascontiguousarray(bv_aug.reshape(1, H * 65))
    bp_row = np.ascontiguousarray(bp[None, :])
    ones_row = np.ones((1, 128), np.float32)

    kbufT = np.zeros((KD, 32), np.float32)
    kbufT[:, :HALO] = buffer[:, :KD].T
    kbuf_h = kbufT.reshape(4, 128, 32).transpose(1, 0, 2).astype(bfl)
    vbuf = np.zeros((32, H * 65), np.float32)
    vb = vbuf.reshape(32, H, 65)
    vb[:HALO, :, :64] = buffer[:, KD:].reshape(HALO, H, DV)
    vb[:HALO, :, 64] = 1.0
    vbuf_h = vbuf.astype(bfl)

    xTp = np.zeros((E, T + HALO + 33), np.float32)
    xTp[:, HALO:HALO + T] = x.T

    in_maps = []
    for c in range(N_CORES):
        t0 = c * SHARD
        xT_c = np.ascontiguousarray(
            xTp[:, t0:t0 + NTOK].reshape(8, 128, NTOK).transpose(1, 0, 2)
        ).astype(bfl)
        mask = np.full((128, NB, 1024), NEG, np.float32)
        for bblk in range(NB):
            i = np.arange(TB)
            tt = t0 + bblk * TB + i
            st = starts[seg[tt]]
            jj = j[tt]
            for r in range(2):
                p = np.arange(128)[:, None]
                g = t0 - HALO + bblk * TB + r * 128 + p
                valid = (
                    (g >= tt[None, :] - HALO) & (g <= tt[None, :])
                    & (g >= st[None, :]) & (g >= 0) & (g < T)
                )
                mask[:, bblk, r * 256:(r + 1) * 256] = np.where(valid, 0.0, NEG)
            p32 = np.arange(32)[:, None]
            g = t0 - HALO + bblk * TB + 256 + p32
            valid = (
                (g >= tt[None, :] - HALO) & (g <= tt[None, :])
                & (g >= st[None, :]) & (g >= 0) & (g < T)
            )
            tailm = np.where(valid, 0.0, NEG)
            pb = np.arange(32)[:, None]
            validb = (pb >= jj[None, :]) & (pb <= HALO - 1)
            bufm = np.where(validb, 0.0, NEG)
            for rr in range(4):
                mask[rr * 32:(rr + 1) * 32, bblk, 512:768] = tailm
                mask[rr * 32:(rr + 1) * 32, bblk, 768:1024] = bufm
        in_maps.append({
            "xT": xT_c, "wk": wk_h, "wq": wq_h, "wv": wv_h, "wp": wp_h,
            "kbufT": kbuf_h, "vbuf": vbuf_h, "ones": ones_row,
            "mask": np.ascontiguousarray(mask),
            "bk": bk2, "bq": bq2, "bv": bv_row, "bp": bp_row,
        })
    return in_maps, seg, j


def kernel(x, Wkv, bkv, Wq, bq, Wp, bp, buffer, sample_lengths):
    in_maps, seg, j = _prep_inputs(
        x, Wkv, bkv, Wq, bq, Wp, bp, buffer, sample_lengths
    )
    with_bias = bool(
        np.any(np.asarray(bkv)) or np.any(np.asarray(bq)) or np.any(np.asarray(bp))
    )
    run = _get_runner(with_bias)
    results = run(in_maps)
    out_full = np.concatenate([results[c]["yout"] for c in range(N_CORES)], axis=0)
    y = np.zeros((B, MAXL, E), np.float32)
    ok = j < MAXL
    y[seg[ok], j[ok]] = out_full[ok]
    return y


# revision 34
# speedup vs baseline: 1.2713x; 1.1556x over previous
"""Trainium2 Bass kernel for nn_DistiledMultiheadAttention_76476187673064.

Sliding-window (W=32) single-query attention over ragged sequences with a
learned pre-context buffer, plus input/output projections.

Strategy (8 NeuronCores, data-parallel over flat tokens):
  - Each core owns 512 tokens; kv for a 31-token halo is recomputed locally
    (plus one masked pad column), so no collectives are needed.
  - All matmul operands are bf16 (host-cast, fp32 PSUM accumulation).
  - HAM-warmth-driven schedule: the attention QK/PV matmuls have low PE
    array activity (32/128 contraction rows resp. 65/128 output columns),
    which keeps the PE clock-gated at 1.2GHz if they run as a contiguous
    phase.  This version interleaves the dense full-array work (V
    projection, output projection, normalization) INTO the attention
    iterations so every HAM activity window sees high utilization and the
    PE stays at 2.4GHz.
  - Startup: x and Wk are split into per-e-chunk DMAs issued concurrently
    from the Sync AND Activation HWDGE queues; K projection runs e-outer
    (4 PSUM accumulators live) so matmuls start as soon as the first
    chunks land instead of after the full tensors.
  - Additive band/segment/buffer masks are host-precomputed in bf16
    (half the HBM traffic); exp without max-subtraction (logits bounded).
  - ScalarE runs ONLY Exp + Copy (single ACT table load); softmax sums
    are gathered per block and reciprocated by one DVE InstReciprocal;
    normalization applied as rank-2 broadcast matmuls into the attention
    output ahead of the output projection.
"""
import math
import sys

sys.path.insert(0, "/opt/trn_rl_repo")

import numpy as np

# ---------------------------------------------------------------- constants
T = 4096
E = 1024
KD = 512          # key dim
H = 16            # heads
W = 32            # window
DK = KD // H      # 32
DV = E // H       # 64
B = 8
MAXL = 768
N_CORES = 8
SHARD = T // N_CORES          # 512 tokens per core
HALO = W - 1                  # 31
NTOK = SHARD + HALO + 1       # 544 token columns incl. halo + 1 pad
TB = 256                      # attention token block
NB = SHARD // TB              # 2 blocks per core
NEG = -30000.0

_CACHE = {}


# ------------------------------------------------------------- tile patches
def _apply_tile_patches():
    """This container's walrus only supports ONE sync-wait per instruction;
    redistribute extra Tile-assigned waits onto single-wait InstNoOp carriers."""
    import concourse.mybir as mybir
    import concourse.tile as tile
    from concourse.vector_clock import ScopedClock

    if getattr(tile.TileContext, "_wait_split_patched", False):
        return
    orig_commit = tile.TileContext._commit_and_lower

    def commit_split(self, inst, original_block, old_bb_map, bb_to_exit_bb):
        si = getattr(inst, "sync_info", None)
        if si is not None and si.on_wait and len(si.on_wait) > 1:
            engine = inst.engine
            if engine is not None and engine != mybir.EngineType.Unassigned:
                waits = list(si.on_wait)
                si.on_wait = waits[-1:]
                for w in waits[:-1]:
                    noop = mybir.InstNoOp(
                        name=self.nc.get_next_instruction_name(),
                        sync_info=mybir.SyncInfo(on_wait=[w], on_update=[]),
                        bass_nofuse=True,
                        engine=engine,
                        text_hint="wait_split",
                    )
                    orig_commit(self, noop, original_block, old_bb_map, bb_to_exit_bb)
        return orig_commit(self, inst, original_block, old_bb_map, bb_to_exit_bb)

    def drain_and_barrier(self, tick_clock, wait_clock):
        drain_inst = self.nc.sync.drain()
        wait_clock.add_sem_waits(
            drain_inst.ins, ScopedClock({None: tick_clock.global_clock})
        )
        si = drain_inst.ins.sync_info
        if si is not None and si.on_wait and len(si.on_wait) > 1:
            waits = list(si.on_wait)
            si.on_wait = waits[:1]
            for w in waits[1:]:
                nop = self.nc.sync.nop(nofuse=True)
                nsi = nop.ins.sync_info
                if nsi is None:
                    nop.ins.sync_info = mybir.SyncInfo(on_wait=[w], on_update=[])
                else:
                    nsi.on_wait = list(nsi.on_wait or []) + [w]
        self.nc.all_engine_barrier()
        assert self.sems is not None
        popped = self.nc._tile_sem_poison_stack.pop()
        assert popped is self._sem_poison
        self.nc.clear_and_free_semaphores(list(self.sems.allocated().values()))
        self.nc.all_engine_barrier()

    tile.TileContext._commit_and_lower = commit_split
    tile.TileContext._drain_and_barrier = drain_and_barrier
    tile.TileContext._wait_split_patched = True


# ------------------------------------------------------------- device build
def _build_nc(with_bias=True):
    import concourse.bass as bass
    import concourse.mybir as mybir
    import concourse.tile as tile

    _apply_tile_patches()
    f32 = mybir.dt.float32
    f32r = mybir.dt.float32r
    bf16 = mybir.dt.bfloat16
    ADD = mybir.AluOpType.add
    MUL = mybir.AluOpType.mult
    EXP = mybir.ActivationFunctionType.Exp

    nc = bass.Bass()
    d_xT = nc.dram_tensor("xT", [128, 8, NTOK], bf16, kind="ExternalInput")
    d_wk = nc.dram_tensor("wk", [128, 8, 4, 128], bf16, kind="ExternalInput")
    d_wq = nc.dram_tensor("wq", [128, 8, 4, 128], bf16, kind="ExternalInput")
    d_wv = nc.dram_tensor("wv", [128, 4, 8, 260], bf16, kind="ExternalInput")
    d_wp = nc.dram_tensor("wp", [128, 2, 8, 512], bf16, kind="ExternalInput")
    d_kbufT = nc.dram_tensor("kbufT", [128, 4, 32], bf16, kind="ExternalInput")
    d_vbuf = nc.dram_tensor("vbuf", [32, 1040], bf16, kind="ExternalInput")
    d_ones = nc.dram_tensor("ones", [1, 128], f32r, kind="ExternalInput")
    d_mask = nc.dram_tensor("mask", [128, NB, 2, 288], f32, kind="ExternalInput")
    d_bk = nc.dram_tensor("bk", [128, 4], f32, kind="ExternalInput")
    d_bq = nc.dram_tensor("bq", [128, 4], f32, kind="ExternalInput")
    d_bv = nc.dram_tensor("bv", [1, H * 65], f32r, kind="ExternalInput")
    d_bp = nc.dram_tensor("bp", [1, E], f32r, kind="ExternalInput")
    d_y = nc.dram_tensor("yout", [SHARD, E], f32, kind="ExternalOutput")
    # sums/recips bounce through DRAM to move data across partitions
    d_rsc = [nc.dram_tensor(f"rscratch{i}", [128, 32], f32, kind="Internal")
             for i in range(NB)]
    d_ssc = [nc.dram_tensor(f"sscratch{i}", [1, 16 * TB], f32, kind="Internal")
             for i in range(NB)]

    with tile.TileContext(nc) as tc, nc.allow_low_precision(
        reason="bf16 matmul operands; fp32 PSUM accumulation throughout"
    ):
        with (
            tc.tile_pool(name="x", bufs=1) as x_pool,
            tc.tile_pool(name="wgt", bufs=1) as w_pool,
            tc.tile_pool(name="const", bufs=1) as const_pool,
            tc.tile_pool(name="kqv", bufs=1) as kqv_pool,
            tc.tile_pool(name="exp", bufs=12) as exp_pool,
            tc.tile_pool(name="srow", bufs=4) as s_pool,
            tc.tile_pool(name="nbc", bufs=3) as nb_pool,
            tc.tile_pool(name="out", bufs=3) as out_pool,
        ):
            # ---- persistent SBUF tiles
            xT = x_pool.tile([128, 8, NTOK], bf16)
            wk_sb = w_pool.tile([128, 8, 4, 128], bf16)
            wq_sb = w_pool.tile([128, 8, 4, 128], bf16)
            wv_sb = w_pool.tile([128, 4, 8, 260], bf16)
            wp_sb = w_pool.tile([128, 2, 8, 512], bf16)
            kbuf_sb = const_pool.tile([128, 4, 32], bf16)
            vbuf_sb = const_pool.tile([128, 1040], bf16)
            ones = const_pool.tile([1, 128], f32r)
            mask_sb = const_pool.tile([128, NB, 2, 288], f32)
            if with_bias:
                bk_sb = const_pool.tile([128, 4], f32)
                bq_sb = const_pool.tile([128, 4], f32)
                bv_sb = const_pool.tile([1, H * 65], f32r)
                bp_sb = const_pool.tile([1, E], f32r)

            # ---- input DMAs, split + spread across both HWDGE engines so
            # K projection can start as soon as the first chunks land.
            # sync: x chunks + everything attention needs early
            # scalar: wk chunks, wv, then late consumers (mask b1, wp)
            for e2 in range(4):
                nc.sync.dma_start(xT[:, 2 * e2:2 * e2 + 2, :],
                                  d_xT[:, 2 * e2:2 * e2 + 2, :])
                nc.scalar.dma_start(wk_sb[:, 2 * e2:2 * e2 + 2, :, :],
                                    d_wk[:, 2 * e2:2 * e2 + 2, :, :])
            nc.sync.dma_start(mask_sb[:, 0, :, :], d_mask[:, 0, :, :])
            nc.sync.dma_start(wq_sb[:], d_wq[:])
            nc.scalar.dma_start(wv_sb[:], d_wv[:])
            nc.sync.dma_start(kbuf_sb[:], d_kbufT[:])
            for r in range(4):
                nc.sync.dma_start(vbuf_sb[r * 32:(r + 1) * 32, :], d_vbuf[:])
            nc.sync.dma_start(ones[:], d_ones[:])
            if with_bias:
                nc.sync.dma_start(bk_sb[:], d_bk[:])
                nc.sync.dma_start(bq_sb[:], d_bq[:])
                nc.sync.dma_start(bv_sb[:], d_bv[:])
                nc.sync.dma_start(bp_sb[:], d_bp[:])
            nc.scalar.dma_start(mask_sb[:, 1, :, :], d_mask[:, 1, :, :])
            nc.scalar.dma_start(wp_sb[:], d_wp[:])

            # ---- persistent activations
            kT = kqv_pool.tile([128, 4, NTOK], bf16)    # K feature-major
            qT = kqv_pool.tile([128, 4, SHARD], bf16)   # Q feature-major (scaled)
            vA = kqv_pool.tile([128, 5, H * 65], bf16)  # V token-major + ones col
            vTail = kqv_pool.tile([128, NB, H * 65], bf16)  # tail V, 4x replicated
            oT = kqv_pool.tile([128, 8, SHARD], bf16)   # attention out feature-major
            # sums/recips: partition p = pair*16 + parity*8 + tc, col = tok%32
            s_half = [kqv_pool.tile([128, 32], f32, name=f"s_half{i}")
                      for i in range(NB)]
            r_half = [kqv_pool.tile([128, 32], f32, name=f"r_half{i}")
                      for i in range(NB)]
            # recips broadcast across partitions (rows 0:64 = even head of
            # the pair, 64:128 = odd), one [128, TB] slab per head pair
            bc_all = [kqv_pool.tile([128, 8, TB], f32, name=f"bc_all{i}")
                      for i in range(NB)]

            # ================= K/Q projection (e-outer so compute starts
            # as soon as chunk 0 of x and Wk arrive)
            with (
                tc.tile_pool(name="ppa", bufs=4, space="PSUM") as ppa,
                tc.tile_pool(name="ppb", bufs=4, space="PSUM") as ppb,
            ):
                pa = [ppa.tile([128, 512], f32, tag="pa", name=f"pa{m}")
                      for m in range(4)]
                pbt = [ppb.tile([128, 32], f32, tag="pb", name=f"pb{m}")
                       for m in range(4)]
                for e in range(8):
                    for m in range(4):
                        nc.tensor.matmul(
                            pa[m][:], wk_sb[:, e, m, :], xT[:, e, 0:512],
                            start=(e == 0), stop=(e == 7),
                        )
                        nc.tensor.matmul(
                            pbt[m][:], wk_sb[:, e, m, :], xT[:, e, 512:NTOK],
                            start=(e == 0), stop=(e == 7),
                        )
                for m in range(4):
                    if with_bias:
                        nc.scalar.add(kT[:, m, 0:512], pa[m][:], bk_sb[:, m:m + 1])
                        nc.scalar.add(kT[:, m, 512:NTOK], pbt[m][:],
                                      bk_sb[:, m:m + 1])
                    elif m % 2 == 0:
                        nc.scalar.copy(kT[:, m, 0:512], pa[m][:])
                        nc.scalar.copy(kT[:, m, 512:NTOK], pbt[m][:])
                    else:
                        nc.vector.tensor_copy(kT[:, m, 0:512], pa[m][:])
                        nc.vector.tensor_copy(kT[:, m, 512:NTOK], pbt[m][:])

                # Q projection (tokens only, no halo)
                qa = [ppa.tile([128, 512], f32, tag="pa", name=f"qa{m}")
                      for m in range(4)]
                for e in range(8):
                    for m in range(4):
                        nc.tensor.matmul(
                            qa[m][:], wq_sb[:, e, m, :], xT[:, e, HALO:HALO + SHARD],
                            start=(e == 0), stop=(e == 7),
                        )
                for m in range(4):
                    if with_bias:
                        nc.scalar.add(qT[:, m, :], qa[m][:], bq_sb[:, m:m + 1])
                    elif m % 2 == 0:
                        nc.scalar.copy(qT[:, m, :], qa[m][:])
                    else:
                        nc.vector.tensor_copy(qT[:, m, :], qa[m][:])

            # ================= attention + V projection + output projection,
            # interleaved so the PE array activity stays high (HAM warm)
            with (
                tc.tile_pool(name="pv", bufs=1, space="PSUM") as pv_pool,
                tc.tile_pool(name="plgM", bufs=4, space="PSUM") as plgM,
                tc.tile_pool(name="plgT", bufs=1, space="PSUM") as plgT,
                tc.tile_pool(name="pov", bufs=2, space="PSUM") as pov,
            ):
                tok_sizes = [128, 128, 128, 128, 32]

                def v_unit(f, i):
                    # V projection token-major (wv pre-augmented with zero
                    # ones-cols): vA[tok, h*65:h*65+65] = [x @ Wv_h.T (+bv) | 1]
                    mt = tok_sizes[i]
                    pvt = pv_pool.tile([128, 260], f32, tag="pv")
                    for e in range(8):
                        nc.tensor.matmul(
                            pvt[0:mt, :],
                            xT[:, e, i * 128:i * 128 + mt],
                            wv_sb[:, f, e, :],
                            start=(e == 0), stop=(e == 7 and not with_bias),
                        )
                    if with_bias:
                        nc.tensor.matmul(
                            pvt[0:mt, :], ones[0:1, 0:mt],
                            bv_sb[0:1, f * 260:(f + 1) * 260],
                            start=False, stop=True,
                        )
                    nc.vector.tensor_copy(
                        vA[0:mt, i, f * 260:(f + 1) * 260], pvt[0:mt, :]
                    )
                    if not with_bias:
                        # ones columns via strided add (psum zeros there)
                        ov_view = vA[0:mt, i, f * 260:(f + 1) * 260].rearrange(
                            "p (h c) -> p h c", c=65
                        )[:, :, 64:65]
                        nc.gpsimd.tensor_scalar_add(ov_view, ov_view, 1.0)

                def vtail_dma(b):
                    # replicate the per-block tail-ctx V rows across all four
                    # 32-partition groups so packed-tail PV matmuls line up
                    for r in range(4):
                        nc.sync.dma_start(
                            vTail[r * 32:(r + 1) * 32, b, :], vA[0:32, 2 * b + 2, :]
                        )

                def emit_chain(b, qi):
                    # reciprocate one quarter-block of softmax sums (2 head
                    # pairs, 32 partitions), bounce through DRAM, and read
                    # back broadcast across partitions into bc_all
                    lo = qi * 32
                    oc = qi * 2
                    nc.sync.dma_start(
                        s_half[b][lo:lo + 32, :],
                        d_ssc[b][0:1, qi * 1024:qi * 1024 + 1024]
                        .rearrange("p (r j) -> (p r) j", j=32),
                    )
                    nc.vector.reciprocal(
                        r_half[b][lo:lo + 32, :], s_half[b][lo:lo + 32, :]
                    )
                    nc.sync.dma_start(
                        d_rsc[b][lo:lo + 32, :], r_half[b][lo:lo + 32, :]
                    )
                    src = d_rsc[b][lo:lo + 32, :].rearrange(
                        "(c q tc) j -> q c tc j", q=2, tc=8
                    )
                    for parity, prow in ((0, 0), (1, 64)):
                        nc.sync.dma_start(
                            bc_all[b][prow:prow + 64, oc:oc + 2, :].rearrange(
                                "p c (tc j) -> p c tc j", j=32
                            ),
                            src[parity:parity + 1].partition_broadcast(64),
                        )

                def norm_unit(b, c):
                    # single DVE multiply against the pre-broadcast recips
                    sl = oT[:, c, b * TB:(b + 1) * TB]
                    nc.vector.tensor_tensor(sl, sl, bc_all[b][:, c, :], MUL)

                def out_unit(b, m, f):
                    # out-projection y[tok, :] = oT.T @ wp (+ bp)
                    pa3 = pov.tile([128, 512], f32, tag="ov")
                    for c in range(8):
                        nc.tensor.matmul(
                            pa3[:], oT[:, c, m * 128:(m + 1) * 128],
                            wp_sb[:, f, c, :], start=(c == 0),
                            stop=(c == 7 and not with_bias),
                        )
                    if with_bias:
                        nc.tensor.matmul(
                            pa3[:], ones[0:1, 0:128],
                            bp_sb[0:1, f * 512:(f + 1) * 512],
                            start=False, stop=True,
                        )
                    ot = out_pool.tile([128, 512], f32)
                    nc.scalar.copy(ot[:], pa3[:])
                    nc.sync.dma_start(
                        d_y[m * 128:(m + 1) * 128, f * 512:(f + 1) * 512], ot[:]
                    )

                def emit_qk(b, g):
                    # band-limited QK: ctx chunk r0 only serves tokens 0:128,
                    # r1 serves 97:256, the tail chunk serves 225:256 — the
                    # masked-out band exterior is never computed.
                    # lg layout: [0:128) = r0-ctx x toks 0:128,
                    #            [128:287) = r1-ctx x toks 97:256
                    base = b * TB
                    lgs = []
                    for hh in range(4):
                        ro = hh * 32
                        lg = plgM.tile([128, 512], f32, tag="lg", name="lg")
                        nc.tensor.matmul(
                            lg[:, 0:128], kT[ro:ro + 32, g, base:base + 128],
                            qT[ro:ro + 32, g, base:base + 128],
                            start=True, stop=True, tile_position=(ro, 0),
                        )
                        lgs.append(lg)
                    exMs = []
                    for hh in range(4):
                        ro = hh * 32
                        nc.tensor.matmul(
                            lgs[hh][:, 128:287],
                            kT[ro:ro + 32, g, base + 128:base + 256],
                            qT[ro:ro + 32, g, base + 97:base + 256],
                            start=True, stop=True, tile_position=(ro, 0),
                        )
                        nc.vector.tensor_tensor(
                            lgs[hh][:, 0:287], lgs[hh][:, 0:287],
                            mask_sb[:, b, 0, 0:287], ADD
                        )
                        ex = exp_pool.tile([128, 287], bf16, tag="ex", name="ex")
                        nc.scalar.activation(ex[:], lgs[hh][:, 0:287], EXP)
                        exMs.append(ex)
                    # lgt layout: [0:31) = tail-ctx x toks 225:256,
                    #             [31:287) = buffer-ctx x toks 0:256
                    lgt = plgT.tile([128, 512], f32, tag="lgt", name="lgt")
                    for hh in range(4):
                        ro = hh * 32
                        nc.tensor.matmul(
                            lgt[ro:ro + 32, 0:31],
                            kT[ro:ro + 32, g, base + 256:base + 288],
                            qT[ro:ro + 32, g, base + 225:base + 256],
                            start=True, stop=True, tile_position=(ro, ro),
                        )
                        nc.tensor.matmul(
                            lgt[ro:ro + 32, 31:287],
                            kbuf_sb[ro:ro + 32, g, :],
                            qT[ro:ro + 32, g, base:base + TB],
                            start=True, stop=True, tile_position=(ro, ro),
                        )
                    nc.vector.tensor_tensor(
                        lgt[:, 0:287], lgt[:, 0:287], mask_sb[:, b, 1, 0:287], ADD
                    )
                    exT = exp_pool.tile([128, 287], bf16, tag="exT", name="exT")
                    nc.scalar.activation(exT[:], lgt[:, 0:287], EXP)
                    return exMs, exT

                def emit_pv(b, g, exMs, exT):
                    # band-limited PV mirrors emit_qk's coverage; one PSUM
                    # accumulation group per head (start marks the whole 2KB
                    # zero region, later col-ranges land on pending zeros)
                    base = b * TB
                    cur_ov = None
                    ods = []
                    for hh in range(4):
                        h = g * 4 + hh
                        q = hh % 2
                        hc = h * 65
                        if q == 0:
                            cur_ov = pov.tile([128, 512], f32, tag="ov")
                            ods = []
                        ovr = cur_ov[:, q * 256:q * 256 + 256]
                        nc.tensor.matmul(
                            ovr[0:65, 0:128], vA[:, 2 * b, hc:hc + 65],
                            exMs[hh][:, 0:128], start=True, stop=False,
                        )
                        nc.tensor.matmul(
                            ovr[0:65, 97:128], vA[:, 2 * b + 1, hc:hc + 65],
                            exMs[hh][:, 128:159], start=False, stop=False,
                        )
                        nc.tensor.matmul(
                            ovr[0:65, 128:256], vA[:, 2 * b + 1, hc:hc + 65],
                            exMs[hh][:, 159:287], start=False, stop=False,
                        )
                        ro = hh * 32
                        nc.tensor.matmul(
                            ovr[0:65, 225:256], vTail[ro:ro + 32, b, hc:hc + 65],
                            exT[ro:ro + 32, 0:31], start=False, stop=False,
                            tile_position=(ro, 0),
                        )
                        nc.tensor.matmul(
                            ovr[0:65, 0:256], vbuf_sb[ro:ro + 32, hc:hc + 65],
                            exT[ro:ro + 32, 31:287], start=False, stop=True,
                            tile_position=(ro, 0),
                        )
                        od = oT[(h % 2) * 64:(h % 2) * 64 + 64, h // 2,
                                base:base + TB]
                        ods.append((od, ovr))
                        if q == 0:
                            # even head drains before the odd head's start
                            # re-marks the shared zero region
                            if hh < 2:
                                nc.scalar.copy(od, ovr[0:64, :])
                            else:
                                nc.vector.tensor_copy(od, ovr[0:64, :])
                        else:
                            # sums first: the recip chain is the kernel tail's
                            # critical path
                            sst = s_pool.tile([1, 2 * TB], f32)
                            if hh < 2:
                                nc.vector.tensor_copy(sst[:], cur_ov[64:65, :])
                            else:
                                nc.scalar.copy(sst[:], cur_ov[64:65, :])
                            pr = h // 2  # head-pair index within the block
                            nc.sync.dma_start(
                                d_ssc[b][0:1, pr * 512:(pr + 1) * 512], sst[:]
                            )
                            od1, ovr1 = ods[1]
                            if hh < 2:
                                nc.scalar.copy(od1, ovr1[0:64, :])
                            else:
                                nc.vector.tensor_copy(od1, ovr1[0:64, :])

                # interleave schedule: pre[it] emitted between QK and PV,
                # post[it] emitted after PV — dense fillers keep HAM warm
                pre = {
                    0: [lambda: v_unit(1, 0), lambda: v_unit(1, 1)],
                    1: [lambda: v_unit(2, 0), lambda: v_unit(2, 1)],
                    2: [lambda: v_unit(3, 0), lambda: v_unit(3, 1)],
                    3: [lambda: v_unit(0, 3), lambda: v_unit(0, 4)],
                    4: [lambda: v_unit(1, 4), lambda: v_unit(2, 4),
                        lambda: v_unit(3, 4), lambda: vtail_dma(1)],
                    5: [lambda: v_unit(1, 3)],
                    6: [lambda: v_unit(2, 3)],
                    7: [lambda: v_unit(3, 3)],
                }
                post = {
                    1: [lambda: emit_chain(0, 0)],
                    2: [lambda: emit_chain(0, 1),
                        lambda: norm_unit(0, 0), lambda: norm_unit(0, 1)],
                    3: [lambda: emit_chain(0, 2),
                        lambda: norm_unit(0, 2), lambda: norm_unit(0, 3)],
                    4: [lambda: emit_chain(0, 3),
                        lambda: norm_unit(0, 4), lambda: norm_unit(0, 5)],
                    5: [lambda: emit_chain(1, 0),
                        lambda: norm_unit(0, 6), lambda: norm_unit(0, 7),
                        lambda: out_unit(0, 0, 0)],
                    6: [lambda: emit_chain(1, 1),
                        lambda: norm_unit(1, 0), lambda: norm_unit(1, 1),
                        lambda: out_unit(0, 0, 1)],
                    7: [lambda: emit_chain(1, 2),
                        lambda: norm_unit(1, 2), lambda: norm_unit(1, 3),
                        lambda: out_unit(0, 1, 0)],
                }

                for f in [lambda: v_unit(0, 0), lambda: v_unit(0, 1),
                          lambda: v_unit(0, 2), lambda: v_unit(1, 2),
                          lambda: v_unit(2, 2), lambda: v_unit(3, 2),
                          lambda: vtail_dma(0)]:
                    f()
                prev = None
                for it in range(NB * 4 + 1):
                    if it < NB * 4:
                        b, g = divmod(it, 4)
                        exMs, exT = emit_qk(b, g)
                        cur = (b, g, exMs, exT)
                    else:
                        cur = None
                    for f in pre.get(it, []):
                        f()
                    if prev is not None:
                        emit_pv(*prev)
                    for f in post.get(it, []):
                        f()
                    prev = cur

                # tail: last recip quarter, remaining norms, and the block-1
                # out-projection with its contraction loop split so most of
                # it runs while the final recips are in flight
                out_unit(0, 1, 1)
                ob = [plgM.tile([128, 512], f32, tag="lg", name=f"ob{u}")
                      for u in range(4)]
                ob_mf = [(2, 0), (2, 1), (3, 0), (3, 1)]
                for c in range(4):
                    for u, (m, f) in enumerate(ob_mf):
                        nc.tensor.matmul(
                            ob[u][:], oT[:, c, m * 128:(m + 1) * 128],
                            wp_sb[:, f, c, :], start=(c == 0), stop=False,
                        )
                norm_unit(1, 4)
                norm_unit(1, 5)
                for c in range(4, 6):
                    for u, (m, f) in enumerate(ob_mf):
                        nc.tensor.matmul(
                            ob[u][:], oT[:, c, m * 128:(m + 1) * 128],
                            wp_sb[:, f, c, :], start=False, stop=False,
                        )
                emit_chain(1, 3)
                norm_unit(1, 6)
                norm_unit(1, 7)
                for c in range(6, 8):
                    for u, (m, f) in enumerate(ob_mf):
                        nc.tensor.matmul(
                            ob[u][:], oT[:, c, m * 128:(m + 1) * 128],
                            wp_sb[:, f, c, :], start=False,
                            stop=(c == 7 and not with_bias),
                        )
                for u, (m, f) in enumerate(ob_mf):
                    if with_bias:
                        nc.tensor.matmul(
                            ob[u][:], ones[0:1, 0:128],
                            bp_sb[0:1, f * 512:(f + 1) * 512],
                            start=False, stop=True,
                        )
                    ot = out_pool.tile([128, 512], f32, name=f"obt{u}")
                    if u % 2 == 0:
                        nc.scalar.copy(ot[:], ob[u][:])
                    else:
                        nc.vector.tensor_copy(ot[:], ob[u][:])
                    nc.sync.dma_start(
                        d_y[m * 128:(m + 1) * 128, f * 512:(f + 1) * 512], ot[:]
                    )
    return nc


def _get_runner(with_bias=True):
    key = ("runner", with_bias)
    if key in _CACHE:
        return _CACHE[key]
    import jax
    import concourse.mybir as mybir
    from concourse import bass2jax
    from jax.sharding import Mesh, PartitionSpec
    from jax.experimental.shard_map import shard_map

    nc = _build_nc(with_bias)
    bass2jax.install_neuronx_cc_hook()
    partition_name = nc.partition_id_tensor.name if nc.partition_id_tensor else None
    in_names, out_names, out_avals, out_shapes = [], [], [], []
    for alloc in nc.m.functions[0].allocations:
        if not isinstance(alloc, mybir.MemoryLocationSet):
            continue
        name = alloc.memorylocations[0].name
        if alloc.kind == "ExternalInput":
            if name != partition_name:
                in_names.append(name)
        elif alloc.kind == "ExternalOutput":
            shape = tuple(alloc.tensor_shape)
            dtype = mybir.dt.np(alloc.dtype)
            out_names.append(name)
            out_avals.append(jax.core.ShapedArray(shape, dtype))
            out_shapes.append((shape, dtype))
    n_params = len(in_names)
    n_outs = len(out_avals)
    all_in_names = in_names + out_names + ([partition_name] if partition_name else [])
    donate = tuple(range(n_params, n_params + n_outs))

    def _body(*args):
        operands = list(args)
        if partition_name is not None:
            operands.append(bass2jax.partition_id_tensor())
        outs = bass2jax._bass_exec_p.bind(
            *operands,
            out_avals=tuple(out_avals),
            in_names=tuple(all_in_names),
            out_names=tuple(out_names),
            lowering_input_output_aliases=(),
            sim_require_finite=True,
            sim_require_nnan=True,
            nc=nc,
        )
        return tuple(outs)

    devices = jax.devices()[:N_CORES]
    mesh = Mesh(np.asarray(devices), ("core",))
    sharded = jax.jit(
        shard_map(
            _body, mesh=mesh,
            in_specs=(PartitionSpec("core"),) * (n_params + n_outs),
            out_specs=(PartitionSpec("core"),) * n_outs,
            check_rep=False,
        ),
        donate_argnums=donate,
        keep_unused=True,
    )

    def run(in_maps):
        per_core = [[np.asarray(m[name]) for name in in_names] for m in in_maps]
        concat_in = [
            np.concatenate([per_core[c][i] for c in range(N_CORES)], axis=0)
            for i in range(n_params)
        ]
        concat_zeros = [
            np.zeros((N_CORES * s[0], *s[1:]), d) for (s, d) in out_shapes
        ]
        out_arrs = sharded(*concat_in, *concat_zeros)
        return [
            {
                name: np.asarray(out_arrs[i]).reshape(N_CORES, *out_shapes[i][0])[c]
                for i, name in enumerate(out_names)
            }
            for c in range(N_CORES)
        ]

    _CACHE[key] = run
    return run


# ------------------------------------------------------------------- host
def _prep_inputs(x, Wkv, bkv, Wq, bq, Wp, bp, buffer, sample_lengths):
    import ml_dtypes

    bfl = ml_dtypes.bfloat16
    x = np.asarray(x, np.float32)
    Wkv = np.asarray(Wkv, np.float32)
    bkv = np.asarray(bkv, np.float32)
    Wq = np.asarray(Wq, np.float32)
    bq = np.asarray(bq, np.float32)
    Wp = np.asarray(Wp, np.float32)
    bp = np.asarray(bp, np.float32)
    buffer = np.asarray(buffer, np.float32)
    lengths = np.asarray(sample_lengths).astype(np.int64)

    scale = 1.0 / math.sqrt(DK)
    starts = np.concatenate([[0], np.cumsum(lengths)[:-1]]).astype(np.int64)
    t = np.arange(T)
    seg = np.searchsorted(starts, t, side="right") - 1
    j = t - starts[seg]

    # weights pre-rearranged into exact SBUF layouts ([p, ...] partition-major)
    wkT = np.ascontiguousarray(Wkv[:KD, :].T)                       # [E, KD]
    wk_h = wkT.reshape(8, 128, 4, 128).transpose(1, 0, 2, 3).astype(bfl)
    wqT = np.ascontiguousarray(Wq.T * scale)                        # [E, KD]
    wq_h = wqT.reshape(8, 128, 4, 128).transpose(1, 0, 2, 3).astype(bfl)
    wv_aug = np.zeros((E, H, 65), np.float32)
    wv_aug[:, :, :64] = Wkv[KD:, :].T.reshape(E, H, DV)
    wv_h = (
        wv_aug.reshape(E, H * 65).reshape(8, 128, 4, 260)
        .transpose(1, 2, 0, 3).astype(bfl)
    )
    wpT = np.ascontiguousarray(Wp.T)                                # [E, E]
    wp_h = wpT.reshape(8, 128, 2, 512).transpose(1, 2, 0, 3).astype(bfl)

    bk2 = np.ascontiguousarray(bkv[:KD].reshape(4, 128).T)
    bq2 = np.ascontiguousarray((bq * scale).reshape(4, 128).T)
    bv_aug = np.zeros((H, 65), np.float32)
    bv_aug[:, :64] = bkv[KD:].reshape(H, DV)
    bv_aug[:, 64] = 1.0
    bv_row = np.ascontiguousarray(bv_aug.reshape(1, H * 65))
    bp_row = np.ascontiguousarray(bp[None, :])
    ones_row = np.ones((1, 128), np.float32)

    kbufT = np.zeros((KD, 32), np.float32)
    kbufT[:, :HALO] = buffer[:, :KD].T
    kbuf_h = kbufT.reshape(4, 128, 32).transpose(1, 0, 2).astype(bfl)
    vbuf = np.zeros((32, H * 65), np.float32)
    vb = vbuf.reshape(32, H, 65)
    vb[:HALO, :, :64] = buffer[:, KD:].reshape(HALO, H, DV)
    vb[:HALO, :, 64] = 1.0
    vbuf_h = vbuf.astype(bfl)

    xTp = np.zeros((E, T + HALO + 33), np.float32)
    xTp[:, HALO:HALO + T] = x.T

    in_maps = []
    for c in range(N_CORES):
        t0 = c * SHARD
        xT_c = np.ascontiguousarray(
            xTp[:, t0:t0 + NTOK].reshape(8, 128, NTOK).transpose(1, 0, 2)
        ).astype(bfl)
        # band-limited mask layout (cols match the banded lg/lgt tiles):
        #   [:, b, 0, 0:128]   r0-ctx  x tokens 0:128
        #   [:, b, 0, 128:287] r1-ctx  x tokens 97:256
        #   [:, b, 1, 0:31]    tail-ctx x tokens 225:256 (4x replicated rows)
        #   [:, b, 1, 31:287]  buffer-ctx x tokens 0:256
        mask = np.full((128, NB, 2, 288), NEG, np.float32)
        p = np.arange(128)[:, None]
        for bblk in range(NB):
            bb = t0 + bblk * TB

            def band(toks, roff, rows):
                tt = bb + toks
                st = starts[seg[tt]]
                g = bb - HALO + roff + rows
                valid = (
                    (g >= tt[None, :] - HALO) & (g <= tt[None, :])
                    & (g >= st[None, :]) & (g >= 0) & (g < T)
                )
                return np.where(valid, 0.0, NEG)

            mask[:, bblk, 0, 0:128] = band(np.arange(128), 0, p)
            mask[:, bblk, 0, 128:287] = band(np.arange(97, 256), 128, p)
            p32 = np.arange(32)[:, None]
            tailm = band(np.arange(225, 256), 256, p32)
            jj = j[bb + np.arange(TB)]
            validb = (p32 >= jj[None, :]) & (p32 <= HALO - 1)
            bufm = np.where(validb, 0.0, NEG)
            for rr in range(4):
                mask[rr * 32:(rr + 1) * 32, bblk, 1, 0:31] = tailm
                mask[rr * 32:(rr + 1) * 32, bblk, 1, 31:287] = bufm
        in_maps.append({
            "xT": xT_c, "wk": wk_h, "wq": wq_h, "wv": wv_h, "wp": wp_h,
            "kbufT": kbuf_h, "vbuf": vbuf_h, "ones": ones_row,
            "mask": np.ascontiguousarray(mask),
            "bk": bk2, "bq": bq2, "bv": bv_row, "bp": bp_row,
        })
    return in_maps, seg, j


def kernel(x, Wkv, bkv, Wq, bq, Wp, bp, buffer, sample_lengths):
    in_maps, seg, j = _prep_inputs(
        x, Wkv, bkv, Wq, bq, Wp, bp, buffer, sample_lengths
    )
    with_bias = bool(
        np.any(np.asarray(bkv)) or np.any(np.asarray(bq)) or np.any(np.asarray(bp))
    )
    run = _get_runner(with_bias)
    results = run(in_maps)
    out_full = np.concatenate([results[c]["yout"] for c in range(N_CORES)], axis=0)
    y = np.zeros((B, MAXL, E), np.float32)
    ok = j < MAXL
    y[seg[ok], j[ok]] = out_full[ok]
    return y


# revision 40
# speedup vs baseline: 1.2957x; 1.0192x over previous
"""Trainium2 Bass kernel for nn_DistiledMultiheadAttention_76476187673064.

Sliding-window (W=32) single-query attention over ragged sequences with a
learned pre-context buffer, plus input/output projections.

Strategy (8 NeuronCores, data-parallel over flat tokens):
  - Each core owns 512 tokens; kv for a 31-token halo is recomputed locally
    (plus one masked pad column), so no collectives are needed.
  - All matmul operands are bf16 (host-cast, fp32 PSUM accumulation).
  - HAM-warmth-driven schedule: the attention QK/PV matmuls have low PE
    array activity (32/128 contraction rows resp. 65/128 output columns),
    which keeps the PE clock-gated at 1.2GHz if they run as a contiguous
    phase.  This version interleaves the dense full-array work (V
    projection, output projection, normalization) INTO the attention
    iterations so every HAM activity window sees high utilization and the
    PE stays at 2.4GHz.
  - Startup: x and Wk are split into per-e-chunk DMAs issued concurrently
    from the Sync AND Activation HWDGE queues; K projection runs e-outer
    (4 PSUM accumulators live) so matmuls start as soon as the first
    chunks land instead of after the full tensors.
  - Additive band/segment/buffer masks are host-precomputed in bf16
    (half the HBM traffic); exp without max-subtraction (logits bounded).
  - ScalarE runs ONLY Exp + Copy (single ACT table load); softmax sums
    are gathered per block and reciprocated by one DVE InstReciprocal;
    normalization applied as rank-2 broadcast matmuls into the attention
    output ahead of the output projection.
"""
import math
import sys

sys.path.insert(0, "/opt/trn_rl_repo")

import numpy as np

# ---------------------------------------------------------------- constants
T = 4096
E = 1024
KD = 512          # key dim
H = 16            # heads
W = 32            # window
DK = KD // H      # 32
DV = E // H       # 64
B = 8
MAXL = 768
N_CORES = 8
SHARD = T // N_CORES          # 512 tokens per core
HALO = W - 1                  # 31
NTOK = SHARD + HALO + 1       # 544 token columns incl. halo + 1 pad
TB = 256                      # attention token block
NB = SHARD // TB              # 2 blocks per core
NEG = -30000.0

_CACHE = {}


# ------------------------------------------------------------- tile patches
def _apply_tile_patches():
    """This container's walrus only supports ONE sync-wait per instruction;
    redistribute extra Tile-assigned waits onto single-wait InstNoOp carriers."""
    import concourse.mybir as mybir
    import concourse.tile as tile
    from concourse.vector_clock import ScopedClock

    if getattr(tile.TileContext, "_wait_split_patched", False):
        return
    orig_commit = tile.TileContext._commit_and_lower

    def commit_split(self, inst, original_block, old_bb_map, bb_to_exit_bb):
        si = getattr(inst, "sync_info", None)
        if si is not None and si.on_wait and len(si.on_wait) > 1:
            engine = inst.engine
            if engine is not None and engine != mybir.EngineType.Unassigned:
                waits = list(si.on_wait)
                si.on_wait = waits[-1:]
                for w in waits[:-1]:
                    noop = mybir.InstNoOp(
                        name=self.nc.get_next_instruction_name(),
                        sync_info=mybir.SyncInfo(on_wait=[w], on_update=[]),
                        bass_nofuse=True,
                        engine=engine,
                        text_hint="wait_split",
                    )
                    orig_commit(self, noop, original_block, old_bb_map, bb_to_exit_bb)
        return orig_commit(self, inst, original_block, old_bb_map, bb_to_exit_bb)

    def drain_and_barrier(self, tick_clock, wait_clock):
        drain_inst = self.nc.sync.drain()
        wait_clock.add_sem_waits(
            drain_inst.ins, ScopedClock({None: tick_clock.global_clock})
        )
        si = drain_inst.ins.sync_info
        if si is not None and si.on_wait and len(si.on_wait) > 1:
            waits = list(si.on_wait)
            si.on_wait = waits[:1]
            for w in waits[1:]:
                nop = self.nc.sync.nop(nofuse=True)
                nsi = nop.ins.sync_info
                if nsi is None:
                    nop.ins.sync_info = mybir.SyncInfo(on_wait=[w], on_update=[])
                else:
                    nsi.on_wait = list(nsi.on_wait or []) + [w]
        self.nc.all_engine_barrier()
        assert self.sems is not None
        popped = self.nc._tile_sem_poison_stack.pop()
        assert popped is self._sem_poison
        self.nc.clear_and_free_semaphores(list(self.sems.allocated().values()))
        self.nc.all_engine_barrier()

    tile.TileContext._commit_and_lower = commit_split
    tile.TileContext._drain_and_barrier = drain_and_barrier
    tile.TileContext._wait_split_patched = True


# ------------------------------------------------------------- device build
def _build_nc(with_bias=True):
    import concourse.bass as bass
    import concourse.mybir as mybir
    import concourse.tile as tile

    _apply_tile_patches()
    f32 = mybir.dt.float32
    f32r = mybir.dt.float32r
    bf16 = mybir.dt.bfloat16
    ADD = mybir.AluOpType.add
    MUL = mybir.AluOpType.mult
    EXP = mybir.ActivationFunctionType.Exp

    nc = bass.Bass()
    d_xT = nc.dram_tensor("xT", [128, 8, NTOK], bf16, kind="ExternalInput")
    d_wk = nc.dram_tensor("wk", [128, 8, 4, 128], bf16, kind="ExternalInput")
    d_wq = nc.dram_tensor("wq", [128, 8, 4, 128], bf16, kind="ExternalInput")
    d_wv = nc.dram_tensor("wv", [128, 4, 8, 260], bf16, kind="ExternalInput")
    d_wp = nc.dram_tensor("wp", [128, 2, 8, 512], bf16, kind="ExternalInput")
    d_kbufT = nc.dram_tensor("kbufT", [128, 4, 32], bf16, kind="ExternalInput")
    d_vbuf = nc.dram_tensor("vbuf", [32, 1040], bf16, kind="ExternalInput")
    d_ones = nc.dram_tensor("ones", [1, 128], f32r, kind="ExternalInput")
    d_mask = nc.dram_tensor("mask", [128, NB, 2, 288], f32, kind="ExternalInput")
    d_bk = nc.dram_tensor("bk", [128, 4], f32, kind="ExternalInput")
    d_bq = nc.dram_tensor("bq", [128, 4], f32, kind="ExternalInput")
    d_bv = nc.dram_tensor("bv", [1, H * 65], f32r, kind="ExternalInput")
    d_bp = nc.dram_tensor("bp", [1, E], f32r, kind="ExternalInput")
    d_y = nc.dram_tensor("yout", [SHARD, E], f32, kind="ExternalOutput")
    # sums/recips bounce through DRAM to move data across partitions
    d_rsc = [nc.dram_tensor(f"rscratch{i}", [128, 32], f32, kind="Internal")
             for i in range(NB)]
    d_ssc = [nc.dram_tensor(f"sscratch{i}", [1, 16 * TB], f32, kind="Internal")
             for i in range(NB)]

    with tile.TileContext(nc) as tc, nc.allow_low_precision(
        reason="bf16 matmul operands; fp32 PSUM accumulation throughout"
    ):
        with (
            tc.tile_pool(name="x", bufs=1) as x_pool,
            tc.tile_pool(name="wgt", bufs=1) as w_pool,
            tc.tile_pool(name="const", bufs=1) as const_pool,
            tc.tile_pool(name="kqv", bufs=1) as kqv_pool,
            tc.tile_pool(name="exp", bufs=12) as exp_pool,
            tc.tile_pool(name="srow", bufs=4) as s_pool,
            tc.tile_pool(name="nbc", bufs=3) as nb_pool,
            tc.tile_pool(name="out", bufs=3) as out_pool,
        ):
            # ---- persistent SBUF tiles
            xT = x_pool.tile([128, 8, NTOK], bf16)
            wk_sb = w_pool.tile([128, 8, 4, 128], bf16)
            wq_sb = w_pool.tile([128, 8, 4, 128], bf16)
            wv_sb = w_pool.tile([128, 4, 8, 260], bf16)
            wp_sb = w_pool.tile([128, 2, 8, 512], bf16)
            kbuf_sb = const_pool.tile([128, 4, 32], bf16)
            vbuf_sb = const_pool.tile([128, 1040], bf16)
            ones = const_pool.tile([1, 128], f32r)
            mask_sb = const_pool.tile([128, NB, 2, 288], f32)
            if with_bias:
                bk_sb = const_pool.tile([128, 4], f32)
                bq_sb = const_pool.tile([128, 4], f32)
                bv_sb = const_pool.tile([1, H * 65], f32r)
                bp_sb = const_pool.tile([1, E], f32r)

            # ---- input DMAs, split + spread across both HWDGE engines so
            # K projection can start as soon as the first chunks land.
            # sync: x chunks + everything attention needs early
            # scalar: wk chunks, wv, then late consumers (mask b1, wp)
            for e2 in range(4):
                nc.sync.dma_start(xT[:, 2 * e2:2 * e2 + 2, :],
                                  d_xT[:, 2 * e2:2 * e2 + 2, :])
                nc.scalar.dma_start(wk_sb[:, 2 * e2:2 * e2 + 2, :, :],
                                    d_wk[:, 2 * e2:2 * e2 + 2, :, :])
            nc.sync.dma_start(mask_sb[:, 0, :, :], d_mask[:, 0, :, :])
            nc.sync.dma_start(wq_sb[:], d_wq[:])
            nc.scalar.dma_start(wv_sb[:], d_wv[:])
            nc.sync.dma_start(kbuf_sb[:], d_kbufT[:])
            for r in range(4):
                nc.sync.dma_start(vbuf_sb[r * 32:(r + 1) * 32, :], d_vbuf[:])
            nc.sync.dma_start(ones[:], d_ones[:])
            if with_bias:
                nc.sync.dma_start(bk_sb[:], d_bk[:])
                nc.sync.dma_start(bq_sb[:], d_bq[:])
                nc.sync.dma_start(bv_sb[:], d_bv[:])
                nc.sync.dma_start(bp_sb[:], d_bp[:])
            nc.scalar.dma_start(mask_sb[:, 1, :, :], d_mask[:, 1, :, :])
            nc.scalar.dma_start(wp_sb[:], d_wp[:])

            # ---- persistent activations
            kT = kqv_pool.tile([128, 4, NTOK], bf16)    # K feature-major
            qT = kqv_pool.tile([128, 4, SHARD], bf16)   # Q feature-major (scaled)
            vA = kqv_pool.tile([128, 5, H * 65], bf16)  # V token-major + ones col
            vTail = kqv_pool.tile([128, NB, H * 65], bf16)  # tail V, 4x replicated
            oT = kqv_pool.tile([128, 8, SHARD], bf16)   # attention out feature-major
            # sums/recips: partition p = pair*16 + parity*8 + tc, col = tok%32
            s_half = [kqv_pool.tile([128, 32], f32, name=f"s_half{i}")
                      for i in range(NB)]
            r_half = [kqv_pool.tile([128, 32], f32, name=f"r_half{i}")
                      for i in range(NB)]
            # recips broadcast across partitions (rows 0:64 = even head of
            # the pair, 64:128 = odd), one [128, TB] slab per head pair
            bc_all = [kqv_pool.tile([128, 8, TB], f32, name=f"bc_all{i}")
                      for i in range(NB)]
            # partition-0 scratch for the tail per-pair chains (DVE needs
            # 32-aligned start partitions)
            sp_t = [kqv_pool.tile([16, 32], f32, name=f"sp_t{i}")
                    for i in range(2)]
            rp_t = [kqv_pool.tile([16, 32], f32, name=f"rp_t{i}")
                    for i in range(2)]

            # ================= K/Q projection (e-outer so compute starts
            # as soon as chunk 0 of x and Wk arrive)
            with (
                tc.tile_pool(name="ppa", bufs=4, space="PSUM") as ppa,
                tc.tile_pool(name="ppb", bufs=4, space="PSUM") as ppb,
            ):
                pa = [ppa.tile([128, 512], f32, tag="pa", name=f"pa{m}")
                      for m in range(4)]
                pbt = [ppb.tile([128, 32], f32, tag="pb", name=f"pb{m}")
                       for m in range(4)]
                for e in range(8):
                    for m in range(4):
                        nc.tensor.matmul(
                            pa[m][:], wk_sb[:, e, m, :], xT[:, e, 0:512],
                            start=(e == 0), stop=(e == 7),
                        )
                        nc.tensor.matmul(
                            pbt[m][:], wk_sb[:, e, m, :], xT[:, e, 512:NTOK],
                            start=(e == 0), stop=(e == 7),
                        )
                for m in range(4):
                    if with_bias:
                        nc.scalar.add(kT[:, m, 0:512], pa[m][:], bk_sb[:, m:m + 1])
                        nc.scalar.add(kT[:, m, 512:NTOK], pbt[m][:],
                                      bk_sb[:, m:m + 1])
                    elif m % 2 == 0:
                        nc.scalar.copy(kT[:, m, 0:512], pa[m][:])
                        nc.scalar.copy(kT[:, m, 512:NTOK], pbt[m][:])
                    else:
                        nc.vector.tensor_copy(kT[:, m, 0:512], pa[m][:])
                        nc.vector.tensor_copy(kT[:, m, 512:NTOK], pbt[m][:])

                # Q projection (tokens only, no halo)
                qa = [ppa.tile([128, 512], f32, tag="pa", name=f"qa{m}")
                      for m in range(4)]
                for e in range(8):
                    for m in range(4):
                        nc.tensor.matmul(
                            qa[m][:], wq_sb[:, e, m, :], xT[:, e, HALO:HALO + SHARD],
                            start=(e == 0), stop=(e == 7),
                        )
                for m in range(4):
                    if with_bias:
                        nc.scalar.add(qT[:, m, :], qa[m][:], bq_sb[:, m:m + 1])
                    elif m % 2 == 0:
                        nc.scalar.copy(qT[:, m, :], qa[m][:])
                    else:
                        nc.vector.tensor_copy(qT[:, m, :], qa[m][:])

            # ================= attention + V projection + output projection,
            # interleaved so the PE array activity stays high (HAM warm)
            with (
                tc.tile_pool(name="pv", bufs=1, space="PSUM") as pv_pool,
                tc.tile_pool(name="plgM", bufs=4, space="PSUM") as plgM,
                tc.tile_pool(name="plgT", bufs=1, space="PSUM") as plgT,
                tc.tile_pool(name="pov", bufs=2, space="PSUM") as pov,
            ):
                tok_sizes = [128, 128, 128, 128, 32]

                def v_unit(f, i):
                    # V projection token-major (wv pre-augmented with zero
                    # ones-cols): vA[tok, h*65:h*65+65] = [x @ Wv_h.T (+bv) | 1]
                    mt = tok_sizes[i]
                    pvt = pv_pool.tile([128, 260], f32, tag="pv")
                    for e in range(8):
                        nc.tensor.matmul(
                            pvt[0:mt, :],
                            xT[:, e, i * 128:i * 128 + mt],
                            wv_sb[:, f, e, :],
                            start=(e == 0), stop=(e == 7 and not with_bias),
                        )
                    if with_bias:
                        nc.tensor.matmul(
                            pvt[0:mt, :], ones[0:1, 0:mt],
                            bv_sb[0:1, f * 260:(f + 1) * 260],
                            start=False, stop=True,
                        )
                    nc.vector.tensor_copy(
                        vA[0:mt, i, f * 260:(f + 1) * 260], pvt[0:mt, :]
                    )
                    if not with_bias:
                        # ones columns via strided add (psum zeros there)
                        ov_view = vA[0:mt, i, f * 260:(f + 1) * 260].rearrange(
                            "p (h c) -> p h c", c=65
                        )[:, :, 64:65]
                        nc.gpsimd.tensor_scalar_add(ov_view, ov_view, 1.0)

                def vtail_dma(b):
                    # replicate the per-block tail-ctx V rows across all four
                    # 32-partition groups so packed-tail PV matmuls line up
                    for r in range(4):
                        nc.sync.dma_start(
                            vTail[r * 32:(r + 1) * 32, b, :], vA[0:32, 2 * b + 2, :]
                        )

                def emit_chain(b, pr0, npr=2):
                    # reciprocate npr head pairs of softmax sums (16 rows per
                    # pair), bounce through DRAM, and read back broadcast
                    # across partitions into bc_all
                    lo = pr0 * 16
                    rows = npr * 16
                    nc.sync.dma_start(
                        s_half[b][lo:lo + rows, :],
                        d_ssc[b][0:1, pr0 * 512:(pr0 + npr) * 512]
                        .rearrange("p (r j) -> (p r) j", j=32),
                    )
                    nc.vector.reciprocal(
                        r_half[b][lo:lo + rows, :], s_half[b][lo:lo + rows, :]
                    )
                    nc.sync.dma_start(
                        d_rsc[b][lo:lo + rows, :], r_half[b][lo:lo + rows, :]
                    )
                    src = d_rsc[b][lo:lo + rows, :].rearrange(
                        "(c q tc) j -> q c tc j", q=2, tc=8
                    )
                    for parity, prow in ((0, 0), (1, 64)):
                        nc.sync.dma_start(
                            bc_all[b][prow:prow + 64, pr0:pr0 + npr, :]
                            .rearrange("p c (tc j) -> p c tc j", j=32),
                            src[parity:parity + 1].partition_broadcast(64),
                        )

                def norm_unit(b, c):
                    # multiply by the pre-broadcast recips on the otherwise
                    # idle GpSimd engine (all operands SBUF)
                    sl = oT[:, c, b * TB:(b + 1) * TB]
                    nc.gpsimd.tensor_tensor(sl, sl, bc_all[b][:, c, :], MUL)

                def out_unit(b, m, f):
                    # out-projection y[tok, :] = oT.T @ wp (+ bp)
                    pa3 = pov.tile([128, 512], f32, tag="ov")
                    for c in range(8):
                        nc.tensor.matmul(
                            pa3[:], oT[:, c, m * 128:(m + 1) * 128],
                            wp_sb[:, f, c, :], start=(c == 0),
                            stop=(c == 7 and not with_bias),
                        )
                    if with_bias:
                        nc.tensor.matmul(
                            pa3[:], ones[0:1, 0:128],
                            bp_sb[0:1, f * 512:(f + 1) * 512],
                            start=False, stop=True,
                        )
                    ot = out_pool.tile([128, 512], f32)
                    nc.scalar.copy(ot[:], pa3[:])
                    nc.sync.dma_start(
                        d_y[m * 128:(m + 1) * 128, f * 512:(f + 1) * 512], ot[:]
                    )

                def emit_qk(b, g):
                    # band-limited QK: ctx chunk r0 only serves tokens 0:128,
                    # r1 serves 97:256, the tail chunk serves 225:256 — the
                    # masked-out band exterior is never computed.
                    # lg layout: [0:128) = r0-ctx x toks 0:128,
                    #            [128:287) = r1-ctx x toks 97:256
                    base = b * TB
                    lgs = []
                    for hh in range(4):
                        ro = hh * 32
                        lg = plgM.tile([128, 512], f32, tag="lg", name="lg")
                        nc.tensor.matmul(
                            lg[:, 0:128], kT[ro:ro + 32, g, base:base + 128],
                            qT[ro:ro + 32, g, base:base + 128],
                            start=True, stop=True, tile_position=(ro, 0),
                        )
                        lgs.append(lg)
                    exMs = []
                    for hh in range(4):
                        ro = hh * 32
                        nc.tensor.matmul(
                            lgs[hh][:, 128:287],
                            kT[ro:ro + 32, g, base + 128:base + 256],
                            qT[ro:ro + 32, g, base + 97:base + 256],
                            start=True, stop=True, tile_position=(ro, 0),
                        )
                        nc.vector.tensor_tensor(
                            lgs[hh][:, 0:287], lgs[hh][:, 0:287],
                            mask_sb[:, b, 0, 0:287], ADD
                        )
                        ex = exp_pool.tile([128, 287], bf16, tag="ex", name="ex")
                        nc.scalar.activation(ex[:], lgs[hh][:, 0:287], EXP)
                        exMs.append(ex)
                    # lgt layout: [0:31) = tail-ctx x toks 225:256,
                    #             [31:287) = buffer-ctx x toks 0:256
                    lgt = plgT.tile([128, 512], f32, tag="lgt", name="lgt")
                    for hh in range(4):
                        ro = hh * 32
                        nc.tensor.matmul(
                            lgt[ro:ro + 32, 0:31],
                            kT[ro:ro + 32, g, base + 256:base + 288],
                            qT[ro:ro + 32, g, base + 225:base + 256],
                            start=True, stop=True, tile_position=(ro, ro),
                        )
                        nc.tensor.matmul(
                            lgt[ro:ro + 32, 31:287],
                            kbuf_sb[ro:ro + 32, g, :],
                            qT[ro:ro + 32, g, base:base + TB],
                            start=True, stop=True, tile_position=(ro, ro),
                        )
                    nc.vector.tensor_tensor(
                        lgt[:, 0:287], lgt[:, 0:287], mask_sb[:, b, 1, 0:287], ADD
                    )
                    exT = exp_pool.tile([128, 287], bf16, tag="exT", name="exT")
                    nc.scalar.activation(exT[:], lgt[:, 0:287], EXP)
                    return exMs, exT

                def emit_pv(b, g, exMs, exT):
                    # band-limited PV mirrors emit_qk's coverage; one PSUM
                    # accumulation group per head (start marks the whole 2KB
                    # zero region, later col-ranges land on pending zeros)
                    base = b * TB
                    cur_ov = None
                    ods = []
                    for hh in range(4):
                        h = g * 4 + hh
                        q = hh % 2
                        hc = h * 65
                        if q == 0:
                            cur_ov = pov.tile([128, 512], f32, tag="ov")
                            ods = []
                        ovr = cur_ov[:, q * 256:q * 256 + 256]
                        nc.tensor.matmul(
                            ovr[0:65, 0:128], vA[:, 2 * b, hc:hc + 65],
                            exMs[hh][:, 0:128], start=True, stop=False,
                        )
                        nc.tensor.matmul(
                            ovr[0:65, 97:128], vA[:, 2 * b + 1, hc:hc + 65],
                            exMs[hh][:, 128:159], start=False, stop=False,
                        )
                        nc.tensor.matmul(
                            ovr[0:65, 128:256], vA[:, 2 * b + 1, hc:hc + 65],
                            exMs[hh][:, 159:287], start=False, stop=False,
                        )
                        ro = hh * 32
                        nc.tensor.matmul(
                            ovr[0:65, 225:256], vTail[ro:ro + 32, b, hc:hc + 65],
                            exT[ro:ro + 32, 0:31], start=False, stop=False,
                            tile_position=(ro, 0),
                        )
                        nc.tensor.matmul(
                            ovr[0:65, 0:256], vbuf_sb[ro:ro + 32, hc:hc + 65],
                            exT[ro:ro + 32, 31:287], start=False, stop=True,
                            tile_position=(ro, 0),
                        )
                        od = oT[(h % 2) * 64:(h % 2) * 64 + 64, h // 2,
                                base:base + TB]
                        ods.append((od, ovr))
                        if q == 0:
                            # even head drains before the odd head's start
                            # re-marks the shared zero region
                            if hh < 2:
                                nc.scalar.copy(od, ovr[0:64, :])
                            else:
                                nc.vector.tensor_copy(od, ovr[0:64, :])
                        else:
                            # sums first: the recip chain is the kernel tail's
                            # critical path
                            sst = s_pool.tile([1, 2 * TB], f32)
                            if hh < 2:
                                nc.vector.tensor_copy(sst[:], cur_ov[64:65, :])
                            else:
                                nc.scalar.copy(sst[:], cur_ov[64:65, :])
                            pr = h // 2  # head-pair index within the block
                            nc.sync.dma_start(
                                d_ssc[b][0:1, pr * 512:(pr + 1) * 512], sst[:]
                            )
                            od1, ovr1 = ods[1]
                            if hh < 2:
                                nc.scalar.copy(od1, ovr1[0:64, :])
                            else:
                                nc.vector.tensor_copy(od1, ovr1[0:64, :])

                # interleave schedule: pre[it] emitted between QK and PV,
                # post[it] emitted after PV — dense fillers keep HAM warm
                pre = {
                    0: [lambda: v_unit(1, 0), lambda: v_unit(1, 1)],
                    1: [lambda: v_unit(2, 0), lambda: v_unit(2, 1)],
                    2: [lambda: v_unit(3, 0), lambda: v_unit(3, 1)],
                    3: [lambda: v_unit(0, 3), lambda: v_unit(0, 4)],
                    4: [lambda: v_unit(1, 4), lambda: v_unit(2, 4),
                        lambda: v_unit(3, 4), lambda: vtail_dma(1)],
                    5: [lambda: v_unit(1, 3)],
                    6: [lambda: v_unit(2, 3)],
                    7: [lambda: v_unit(3, 3)],
                }
                post = {
                    1: [lambda: emit_chain(0, 0)],
                    2: [lambda: emit_chain(0, 2),
                        lambda: norm_unit(0, 0), lambda: norm_unit(0, 1)],
                    3: [lambda: emit_chain(0, 4),
                        lambda: norm_unit(0, 2), lambda: norm_unit(0, 3)],
                    4: [lambda: emit_chain(0, 6),
                        lambda: norm_unit(0, 4), lambda: norm_unit(0, 5)],
                    5: [lambda: emit_chain(1, 0),
                        lambda: norm_unit(0, 6), lambda: norm_unit(0, 7),
                        lambda: out_unit(0, 0, 0)],
                    6: [lambda: emit_chain(1, 2),
                        lambda: norm_unit(1, 0), lambda: norm_unit(1, 1),
                        lambda: out_unit(0, 0, 1)],
                    7: [lambda: emit_chain(1, 4),
                        lambda: norm_unit(1, 2), lambda: norm_unit(1, 3),
                        lambda: out_unit(0, 1, 0)],
                }

                for f in [lambda: v_unit(0, 0), lambda: v_unit(0, 1),
                          lambda: v_unit(0, 2), lambda: v_unit(1, 2),
                          lambda: v_unit(2, 2), lambda: v_unit(3, 2),
                          lambda: vtail_dma(0)]:
                    f()
                prev = None
                for it in range(NB * 4 + 1):
                    if it < NB * 4:
                        b, g = divmod(it, 4)
                        exMs, exT = emit_qk(b, g)
                        cur = (b, g, exMs, exT)
                    else:
                        cur = None
                    for f in pre.get(it, []):
                        f()
                    if prev is not None:
                        emit_pv(*prev)
                    for f in post.get(it, []):
                        f()
                    prev = cur

                def emit_chain_pair_tail(pr, scr):
                    # single-pair chain via partition-0 scratch
                    nc.sync.dma_start(
                        sp_t[scr][:],
                        d_ssc[1][0:1, pr * 512:(pr + 1) * 512]
                        .rearrange("p (r j) -> (p r) j", j=32),
                    )
                    nc.vector.reciprocal(rp_t[scr][:], sp_t[scr][:])
                    nc.sync.dma_start(
                        d_rsc[1][pr * 16:(pr + 1) * 16, :], rp_t[scr][:]
                    )
                    src = d_rsc[1][pr * 16:(pr + 1) * 16, :].rearrange(
                        "(c q tc) j -> q c tc j", q=2, tc=8
                    )
                    for parity, prow in ((0, 0), (1, 64)):
                        nc.sync.dma_start(
                            bc_all[1][prow:prow + 64, pr:pr + 1, :]
                            .rearrange("p c (tc j) -> p c tc j", j=32),
                            src[parity:parity + 1].partition_broadcast(64),
                        )

                # tail: per-pair recip chains for the last two pairs, the
                # remaining norms, and the block-1 out-projection with its
                # contraction loop split so most of it runs while the final
                # recips are in flight
                emit_chain_pair_tail(6, 0)
                emit_chain_pair_tail(7, 1)
                out_unit(0, 1, 1)
                ob = [plgM.tile([128, 512], f32, tag="lg", name=f"ob{u}")
                      for u in range(4)]
                ob_mf = [(2, 0), (2, 1), (3, 0), (3, 1)]
                for c in range(4):
                    for u, (m, f) in enumerate(ob_mf):
                        nc.tensor.matmul(
                            ob[u][:], oT[:, c, m * 128:(m + 1) * 128],
                            wp_sb[:, f, c, :], start=(c == 0), stop=False,
                        )
                norm_unit(1, 4)
                norm_unit(1, 5)
                for c in range(4, 6):
                    for u, (m, f) in enumerate(ob_mf):
                        nc.tensor.matmul(
                            ob[u][:], oT[:, c, m * 128:(m + 1) * 128],
                            wp_sb[:, f, c, :], start=False, stop=False,
                        )
                norm_unit(1, 6)
                for u, (m, f) in enumerate(ob_mf):
                    nc.tensor.matmul(
                        ob[u][:], oT[:, 6, m * 128:(m + 1) * 128],
                        wp_sb[:, f, 6, :], start=False, stop=False,
                    )
                norm_unit(1, 7)
                for u, (m, f) in enumerate(ob_mf):
                    nc.tensor.matmul(
                        ob[u][:], oT[:, 7, m * 128:(m + 1) * 128],
                        wp_sb[:, f, 7, :], start=False, stop=not with_bias,
                    )
                for u, (m, f) in enumerate(ob_mf):
                    if with_bias:
                        nc.tensor.matmul(
                            ob[u][:], ones[0:1, 0:128],
                            bp_sb[0:1, f * 512:(f + 1) * 512],
                            start=False, stop=True,
                        )
                    ot = out_pool.tile([128, 512], f32, name=f"obt{u}")
                    if u % 2 == 0:
                        nc.scalar.copy(ot[:], ob[u][:])
                    else:
                        nc.vector.tensor_copy(ot[:], ob[u][:])
                    nc.sync.dma_start(
                        d_y[m * 128:(m + 1) * 128, f * 512:(f + 1) * 512], ot[:]
                    )
    return nc


def _get_runner(with_bias=True):
    key = ("runner", with_bias)
    if key in _CACHE:
        return _CACHE[key]
    import jax
    import concourse.mybir as mybir
    from concourse import bass2jax
    from jax.sharding import Mesh, PartitionSpec
    from jax.experimental.shard_map import shard_map

    nc = _build_nc(with_bias)
    bass2jax.install_neuronx_cc_hook()
    partition_name = nc.partition_id_tensor.name if nc.partition_id_tensor else None
    in_names, out_names, out_avals, out_shapes = [], [], [], []
    for alloc in nc.m.functions[0].allocations:
        if not isinstance(alloc, mybir.MemoryLocationSet):
            continue
        name = alloc.memorylocations[0].name
        if alloc.kind == "ExternalInput":
            if name != partition_name:
                in_names.append(name)
        elif alloc.kind == "ExternalOutput":
            shape = tuple(alloc.tensor_shape)
            dtype = mybir.dt.np(alloc.dtype)
            out_names.append(name)
            out_avals.append(jax.core.ShapedArray(shape, dtype))
            out_shapes.append((shape, dtype))
    n_params = len(in_names)
    n_outs = len(out_avals)
    all_in_names = in_names + out_names + ([partition_name] if partition_name else [])
    donate = tuple(range(n_params, n_params + n_outs))

    def _body(*args):
        operands = list(args)
        if partition_name is not None:
            operands.append(bass2jax.partition_id_tensor())
        outs = bass2jax._bass_exec_p.bind(
            *operands,
            out_avals=tuple(out_avals),
            in_names=tuple(all_in_names),
            out_names=tuple(out_names),
            lowering_input_output_aliases=(),
            sim_require_finite=True,
            sim_require_nnan=True,
            nc=nc,
        )
        return tuple(outs)

    devices = jax.devices()[:N_CORES]
    mesh = Mesh(np.asarray(devices), ("core",))
    sharded = jax.jit(
        shard_map(
            _body, mesh=mesh,
            in_specs=(PartitionSpec("core"),) * (n_params + n_outs),
            out_specs=(PartitionSpec("core"),) * n_outs,
            check_rep=False,
        ),
        donate_argnums=donate,
        keep_unused=True,
    )

    def run(in_maps):
        per_core = [[np.asarray(m[name]) for name in in_names] for m in in_maps]
        concat_in = [
            np.concatenate([per_core[c][i] for c in range(N_CORES)], axis=0)
            for i in range(n_params)
        ]
        concat_zeros = [
            np.zeros((N_CORES * s[0], *s[1:]), d) for (s, d) in out_shapes
        ]
        out_arrs = sharded(*concat_in, *concat_zeros)
        return [
            {
                name: np.asarray(out_arrs[i]).reshape(N_CORES, *out_shapes[i][0])[c]
                for i, name in enumerate(out_names)
            }
            for c in range(N_CORES)
        ]

    _CACHE[key] = run
    return run


# ------------------------------------------------------------------- host
def _prep_inputs(x, Wkv, bkv, Wq, bq, Wp, bp, buffer, sample_lengths):
    import ml_dtypes

    bfl = ml_dtypes.bfloat16
    x = np.asarray(x, np.float32)
    Wkv = np.asarray(Wkv, np.float32)
    bkv = np.asarray(bkv, np.float32)
    Wq = np.asarray(Wq, np.float32)
    bq = np.asarray(bq, np.float32)
    Wp = np.asarray(Wp, np.float32)
    bp = np.asarray(bp, np.float32)
    buffer = np.asarray(buffer, np.float32)
    lengths = np.asarray(sample_lengths).astype(np.int64)

    scale = 1.0 / math.sqrt(DK)
    starts = np.concatenate([[0], np.cumsum(lengths)[:-1]]).astype(np.int64)
    t = np.arange(T)
    seg = np.searchsorted(starts, t, side="right") - 1
    j = t - starts[seg]

    # weights pre-rearranged into exact SBUF layouts ([p, ...] partition-major)
    wkT = np.ascontiguousarray(Wkv[:KD, :].T)                       # [E, KD]
    wk_h = wkT.reshape(8, 128, 4, 128).transpose(1, 0, 2, 3).astype(bfl)
    wqT = np.ascontiguousarray(Wq.T * scale)                        # [E, KD]
    wq_h = wqT.reshape(8, 128, 4, 128).transpose(1, 0, 2, 3).astype(bfl)
    wv_aug = np.zeros((E, H, 65), np.float32)
    wv_aug[:, :, :64] = Wkv[KD:, :].T.reshape(E, H, DV)
    wv_h = (
        wv_aug.reshape(E, H * 65).reshape(8, 128, 4, 260)
        .transpose(1, 2, 0, 3).astype(bfl)
    )
    wpT = np.ascontiguousarray(Wp.T)                                # [E, E]
    wp_h = wpT.reshape(8, 128, 2, 512).transpose(1, 2, 0, 3).astype(bfl)

    bk2 = np.ascontiguousarray(bkv[:KD].reshape(4, 128).T)
    bq2 = np.ascontiguousarray((bq * scale).reshape(4, 128).T)
    bv_aug = np.zeros((H, 65), np.float32)
    bv_aug[:, :64] = bkv[KD:].reshape(H, DV)
    bv_aug[:, 64] = 1.0
    bv_row = np.ascontiguousarray(bv_aug.reshape(1, H * 65))
    bp_row = np.ascontiguousarray(bp[None, :])
    ones_row = np.ones((1, 128), np.float32)

    kbufT = np.zeros((KD, 32), np.float32)
    kbufT[:, :HALO] = buffer[:, :KD].T
    kbuf_h = kbufT.reshape(4, 128, 32).transpose(1, 0, 2).astype(bfl)
    vbuf = np.zeros((32, H * 65), np.float32)
    vb = vbuf.reshape(32, H, 65)
    vb[:HALO, :, :64] = buffer[:, KD:].reshape(HALO, H, DV)
    vb[:HALO, :, 64] = 1.0
    vbuf_h = vbuf.astype(bfl)

    xTp = np.zeros((E, T + HALO + 33), np.float32)
    xTp[:, HALO:HALO + T] = x.T

    in_maps = []
    for c in range(N_CORES):
        t0 = c * SHARD
        xT_c = np.ascontiguousarray(
            xTp[:, t0:t0 + NTOK].reshape(8, 128, NTOK).transpose(1, 0, 2)
        ).astype(bfl)
        # band-limited mask layout (cols match the banded lg/lgt tiles):
        #   [:, b, 0, 0:128]   r0-ctx  x tokens 0:128
        #   [:, b, 0, 128:287] r1-ctx  x tokens 97:256
        #   [:, b, 1, 0:31]    tail-ctx x tokens 225:256 (4x replicated rows)
        #   [:, b, 1, 31:287]  buffer-ctx x tokens 0:256
        mask = np.full((128, NB, 2, 288), NEG, np.float32)
        p = np.arange(128)[:, None]
        for bblk in range(NB):
            bb = t0 + bblk * TB

            def band(toks, roff, rows):
                tt = bb + toks
                st = starts[seg[tt]]
                g = bb - HALO + roff + rows
                valid = (
                    (g >= tt[None, :] - HALO) & (g <= tt[None, :])
                    & (g >= st[None, :]) & (g >= 0) & (g < T)
                )
                return np.where(valid, 0.0, NEG)

            mask[:, bblk, 0, 0:128] = band(np.arange(128), 0, p)
            mask[:, bblk, 0, 128:287] = band(np.arange(97, 256), 128, p)
            p32 = np.arange(32)[:, None]
            tailm = band(np.arange(225, 256), 256, p32)
            jj = j[bb + np.arange(TB)]
            validb = (p32 >= jj[None, :]) & (p32 <= HALO - 1)
            bufm = np.where(validb, 0.0, NEG)
            for rr in range(4):
                mask[rr * 32:(rr + 1) * 32, bblk, 1, 0:31] = tailm
                mask[rr * 32:(rr + 1) * 32, bblk, 1, 31:287] = bufm
        in_maps.append({
            "xT": xT_c, "wk": wk_h, "wq": wq_h, "wv": wv_h, "wp": wp_h,
            "kbufT": kbuf_h, "vbuf": vbuf_h, "ones": ones_row,
            "mask": np.ascontiguousarray(mask),
            "bk": bk2, "bq": bq2, "bv": bv_row, "bp": bp_row,
        })
    return in_maps, seg, j


def kernel(x, Wkv, bkv, Wq, bq, Wp, bp, buffer, sample_lengths):
    in_maps, seg, j = _prep_inputs(
        x, Wkv, bkv, Wq, bq, Wp, bp, buffer, sample_lengths
    )
    with_bias = bool(
        np.any(np.asarray(bkv)) or np.any(np.asarray(bq)) or np.any(np.asarray(bp))
    )
    run = _get_runner(with_bias)
    results = run(in_maps)
    out_full = np.concatenate([results[c]["yout"] for c in range(N_CORES)], axis=0)
    y = np.zeros((B, MAXL, E), np.float32)
    ok = j < MAXL
    y[seg[ok], j[ok]] = out_full[ok]
    return y
